# revision 1
# baseline (speedup 1.0000x reference)
"""Trainium2 Bass kernel for nn_MistralMoLoraLayer (MoE-routed LoRA FFN).

Strategy: data-parallel over tokens (8 cores x 256 tokens), base FFN weights
replicated, all-expert LoRA replicated. The per-(batch,slot) softmax over the
sequence axis needs global denominators -> tiny [2,8] AllReduce.

Per-core math (all tiles [h/er/d partitions, tokens free]):
  router: logits = x @ gate_w.T; top-2 (value,index) per token; exp; AR of
          per-batch-slot sums; weights w_j = exp_j / denom[batch, j]
  A-proj: UA/GA [E*R=128, t] = stacked up_A/gate_A @ x.T   (one K=128 chain)
  slot-mask trick: Ut_j = UA * M_j where M_j[e*R+r, t] = (sel_j(t)==e);
          lo_up_j[h,t] = (stacked up_B) @ Ut_j  == up_B[sel_j(t)] @ u_{sel_j(t)}
  h_j = silu(U + lo_up_j) * (G + lo_gate_j); ch_j = c_j * h_j
  mixed = ch_0 + ch_1
  v_j[er,t] = (stacked down_A) @ ch_j  (accumulated over h), masked by M_j
  outT[d,t] = w_down-chain @ mixed + (stacked down_B) @ v_0 + ... @ v_1
"""

import numpy as np

# problem constants (hardcoded; kernel.py must be self-contained)
B, S, D, H, E, R, TOPK = 2, 1024, 2048, 5632, 8, 16, 2
ALPHA = 2.0
T = B * S
NCORES = 8
TC = T // NCORES           # 256 tokens per core
KT = D // 128              # 16 k-tiles over D
HT = H // 128              # 44 h-tiles
DT = D // 128              # 16 d-tiles
ER = E * R                 # 128

MM_MODE = "hyb"            # "f32" | "f32r" | "bf16" | "hyb"
DEBUG_TAPS = False         # add intermediate-tensor outputs for debugging
SKIP_AR = False            # replace AllReduce with local copy (for TimelineSim)

_cache = {}


def _np_sd():
    import ml_dtypes
    return np.dtype(ml_dtypes.bfloat16) if MM_MODE == "bf16" else np.dtype(np.float32)


def _build():
    import concourse.bacc as bacc
    import concourse.bass as bass
    import concourse.mybir as mybir
    import concourse.tile as tile
    from concourse.masks import make_identity

    f32 = mybir.dt.float32
    bf16 = mybir.dt.bfloat16
    SD = bf16 if MM_MODE == "bf16" else f32
    WUG = bf16 if MM_MODE in ("bf16", "hyb") else f32  # up/gate weights + x
    AL = mybir.AluOpType
    AF = mybir.ActivationFunctionType

    def mm(ap):
        # matmul operand dtype override for f32r-path tensors
        if MM_MODE in ("f32r", "hyb"):
            return ap.bitcast(mybir.dt.float32r)
        return ap

    mo = mm  # producer outputs feeding f32r matmuls must also be f32r-typed

    def mug(ap):
        # up/gate-path operands: true bf16 in hyb/bf16, f32r in f32r mode
        if MM_MODE == "f32r":
            return ap.bitcast(mybir.dt.float32r)
        return ap

    nc = bacc.Bacc("TRN2", target_bir_lowering=False, debug=False,
                   num_devices=NCORES)

    # ---- DRAM I/O ----
    d_xT = nc.dram_tensor("xT", [D, TC], SD, kind="ExternalInput").ap()
    if MM_MODE == "bf16":
        d_xTr = nc.dram_tensor("xTr", [D, TC], f32, kind="ExternalInput").ap()
    else:
        d_xTr = d_xT
    d_gw = nc.dram_tensor("gw", [128, KT * E], f32, kind="ExternalInput").ap()
    d_wu = nc.dram_tensor("wu", [HT, 128, KT * 128], WUG, kind="ExternalInput").ap()
    d_wg = nc.dram_tensor("wg", [HT, 128, KT * 128], WUG, kind="ExternalInput").ap()
    d_wd = nc.dram_tensor("wd", [DT, 128, HT * 128], SD, kind="ExternalInput").ap()
    d_A = nc.dram_tensor("Ah", [128, KT * 2 * ER], SD, kind="ExternalInput").ap()
    d_uB = nc.dram_tensor("uB", [HT, 128, 128], SD, kind="ExternalInput").ap()
    d_gB = nc.dram_tensor("gB", [HT, 128, 128], SD, kind="ExternalInput").ap()
    d_dA = nc.dram_tensor("dA", [HT, 128, ER], SD, kind="ExternalInput").ap()
    d_dB = nc.dram_tensor("dB", [128, D], SD, kind="ExternalInput").ap()
    d_eid = nc.dram_tensor("eid", [128, 1], f32, kind="ExternalInput").ap()
    d_i8m = nc.dram_tensor("i8m", [128, E], f32, kind="ExternalInput").ap()
    d_bsr = nc.dram_tensor("bsr", [1, 2], f32, kind="ExternalInput").ap()
    d_bsc = nc.dram_tensor("bsc", [2, 1], f32, kind="ExternalInput").ap()
    d_sel2 = nc.dram_tensor("sel2", [2, 256], f32, kind="ExternalInput").ap()
    d_out = nc.dram_tensor("outT", [D, TC], f32, kind="ExternalOutput").ap()

    with tile.TileContext(nc) as tc:
        import contextlib
        ctx = contextlib.ExitStack()
        with ctx:
            cpool = ctx.enter_context(tc.tile_pool(name="const", bufs=1))
            wpool = ctx.enter_context(tc.tile_pool(name="wstream", bufs=2))
            bpool = ctx.enter_context(tc.tile_pool(name="bstream", bufs=3))
            spool = ctx.enter_context(tc.tile_pool(name="work", bufs=2))
            pspool = ctx.enter_context(
                tc.tile_pool(name="ps", bufs=1, space="PSUM"))
            drpool = ctx.enter_context(
                tc.tile_pool(name="dram", bufs=1, space="DRAM"))

            # ---- constants / resident tiles ----
            xT_sb = cpool.tile([128, KT * TC], SD, name="xT_sb")
            for k in range(KT):
                nc.sync.dma_start(out=mo(xT_sb[:, k * TC:(k + 1) * TC]),
                                  in_=mo(d_xT[k * 128:(k + 1) * 128, :]))
            if MM_MODE == "bf16":
                xTr_sb = cpool.tile([128, KT * TC], f32, name="xTr_sb")
                for k in range(KT):
                    nc.sync.dma_start(out=xTr_sb[:, k * TC:(k + 1) * TC],
                                      in_=d_xTr[k * 128:(k + 1) * 128, :])
            elif MM_MODE in ("f32r", "hyb"):
                xTr_sb = xT_sb.bitcast(f32)   # same bits, f32 view for router
            else:
                xTr_sb = xT_sb
            if MM_MODE == "hyb":
                # bf16 copy of x for the up/gate base GEMMs (gpsimd casts)
                xTb = cpool.tile([128, KT * TC], bf16, name="xTb")
                for k in range(KT):
                    nc.gpsimd.dma_start(out=xTb[:, k * TC:(k + 1) * TC],
                                        in_=d_xT[k * 128:(k + 1) * 128, :])
            else:
                xTb = xT_sb
            A_sb = cpool.tile([128, KT * 2 * ER], SD, name="A_sb")
            nc.sync.dma_start(out=mo(A_sb[:]), in_=mo(d_A[:]))
            dB_sb = cpool.tile([128, D], SD, name="dB_sb")
            nc.sync.dma_start(out=mo(dB_sb[:]), in_=mo(d_dB[:]))
            gw_sb = cpool.tile([128, KT * E], f32, name="gw_sb")
            nc.sync.dma_start(out=gw_sb[:], in_=d_gw[:])
            eid_sb = cpool.tile([128, 1], f32, name="eid_sb")
            nc.sync.dma_start(out=eid_sb[:], in_=d_eid[:])
            i8m_sb = cpool.tile([128, E], f32, name="i8m_sb")
            nc.sync.dma_start(out=i8m_sb[:], in_=d_i8m[:])
            bsr_sb = cpool.tile([1, 2], f32, name="bsr_sb")
            nc.sync.dma_start(out=bsr_sb[:], in_=d_bsr[:])
            bsc_sb = cpool.tile([2, 1], f32, name="bsc_sb")
            nc.sync.dma_start(out=bsc_sb[:], in_=d_bsc[:])
            sel2_sb = cpool.tile([2, 256], f32, name="sel2_sb")
            nc.sync.dma_start(out=sel2_sb[:], in_=d_sel2[:])

            ident = cpool.tile([128, 128], f32, name="ident")
            make_identity(nc, ident)
            ones_row = cpool.tile([1, 128], f32, name="ones_row")
            nc.vector.memset(ones_row, 1.0)
            ones_col = cpool.tile([128, 1], f32, name="ones_col")
            nc.vector.memset(ones_col, 1.0)

            mixed = cpool.tile([128, HT * TC], SD, name="mixed")
            ev_rows = cpool.tile([2, TC], f32, name="ev_rows")
            s_rows = cpool.tile([2, TC], f32, name="s_rows")
            crows = cpool.tile([2, TC], f32, name="crows")
            cb = cpool.tile([128, 2 * TC], SD, name="cb")
            Mj = cpool.tile([128, 2 * TC], SD, name="Mj")
            UA = cpool.tile([128, TC], SD, name="UA")
            GA = cpool.tile([128, TC], SD, name="GA")
            Ut = cpool.tile([128, 2 * TC], SD, name="Ut")
            Gt = cpool.tile([128, 2 * TC], SD, name="Gt")
            vt = cpool.tile([128, 2 * TC], SD, name="vt")

            # ---- phase 1: router ----
            den_parts = cpool.tile([1, 4], f32, name="den_parts")
            for tt in range(2):
                psL = pspool.tile([128, TC], f32, tag="ps_small", name="psL")
                for k in range(KT):
                    nc.tensor.matmul(
                        psL[:, 0:E],
                        xTr_sb[:, k * TC + tt * 128: k * TC + tt * 128 + 128],
                        gw_sb[:, k * E:(k + 1) * E],
                        start=(k == 0), stop=(k == KT - 1))
                L = spool.tile([128, E], f32, tag="L")
                nc.vector.tensor_copy(L[:], psL[:, 0:E])
                mx1 = spool.tile([128, 1], f32, tag="mx1")
                nc.vector.tensor_reduce(mx1[:], L[:], mybir.AxisListType.X, AL.max)
                msk = spool.tile([128, E], f32, tag="msk")
                nc.vector.tensor_scalar(msk[:], L[:], mx1[:], None, AL.is_equal)
                mi = spool.tile([128, E], f32, tag="mi")
                nc.vector.tensor_tensor(mi[:], msk[:], i8m_sb[:], AL.mult)
                svals = spool.tile([128, 2], f32, tag="svals")
                nc.vector.tensor_reduce(svals[:, 0:1], mi[:],
                                        mybir.AxisListType.X, AL.max)
                evals = spool.tile([128, 2], f32, tag="evals")
                nc.scalar.activation(evals[:, 0:1], mx1[:], AF.Exp)
                # mask out slot-0 winner, find second max
                big = spool.tile([128, E], f32, tag="big")
                nc.vector.tensor_scalar(big[:], msk[:], 1e30, None, AL.mult)
                L2 = spool.tile([128, E], f32, tag="L2")
                nc.vector.tensor_tensor(L2[:], L[:], big[:], AL.subtract)
                mx2 = spool.tile([128, 1], f32, tag="mx2")
                nc.vector.tensor_reduce(mx2[:], L2[:], mybir.AxisListType.X, AL.max)
                msk2 = spool.tile([128, E], f32, tag="msk2")
                nc.vector.tensor_scalar(msk2[:], L2[:], mx2[:], None, AL.is_equal)
                mi2 = spool.tile([128, E], f32, tag="mi2")
                nc.vector.tensor_tensor(mi2[:], msk2[:], i8m_sb[:], AL.mult)
                nc.vector.tensor_reduce(svals[:, 1:2], mi2[:],
                                        mybir.AxisListType.X, AL.max)
                nc.scalar.activation(evals[:, 1:2], mx2[:], AF.Exp)
                # per-tile partial denominators: [1,2] = ones.T @ evals
                psd = pspool.tile([1, 2], f32, tag="ps_small", name="psd")
                nc.tensor.matmul(psd[:], ones_col[:], evals[:],
                                 start=True, stop=True)
                nc.vector.tensor_copy(den_parts[:, tt * 2:(tt + 1) * 2], psd[:])
                # transpose evals/svals -> rows
                psT = pspool.tile([2, 128], f32, tag="ps_small", name="psT")
                nc.tensor.transpose(psT[:], evals[:], ident[:])
                nc.vector.tensor_copy(ev_rows[:, tt * 128:(tt + 1) * 128], psT[:])
                psT2 = pspool.tile([2, 128], f32, tag="ps_small", name="psT2")
                nc.tensor.transpose(psT2[:], svals[:], ident[:])
                nc.vector.tensor_copy(s_rows[:, tt * 128:(tt + 1) * 128], psT2[:])

            # combine partials, AllReduce [2,8] (row b = batch, cols 0:2 used)
            drow = cpool.tile([1, 2], f32, name="drow")
            nc.vector.tensor_tensor(drow[:], den_parts[:, 0:2],
                                    den_parts[:, 2:4], AL.add)
            ar_sb = cpool.tile([2, 8], f32, name="ar_sb")
            nc.vector.memset(ar_sb, 0.0)
            psAR = pspool.tile([2, 2], f32, tag="ps_small", name="psAR")
            nc.tensor.matmul(psAR[:], bsr_sb[:], drow[:], start=True, stop=True)
            nc.vector.tensor_copy(ar_sb[:, 0:2], psAR[:])
            ar_in = drpool.tile([2, 8], f32, name="ar_in")
            ar_out = drpool.tile([2, 8], f32, name="ar_out", addr_space="Shared")
            nc.gpsimd.dma_start(out=ar_in[:], in_=ar_sb[:])
            if SKIP_AR:
                nc.gpsimd.dma_start(out=ar_out[:], in_=ar_in[:])
            else:
                nc.gpsimd.collective_compute(
                    "AllReduce", AL.add,
                    replica_groups=[list(range(NCORES))],
                    ins=[ar_in.opt()], outs=[ar_out.opt()])
            den2 = cpool.tile([2, 8], f32, name="den2")
            nc.gpsimd.dma_start(out=den2[:], in_=ar_out[:])
            # select this core's batch row -> [2(slots),1], reciprocal
            psDC = pspool.tile([2, 1], f32, tag="ps_small", name="psDC")
            nc.tensor.matmul(psDC[:], den2[:, 0:2], bsc_sb[:],
                             start=True, stop=True)
            rcp = cpool.tile([2, 1], f32, name="rcp")
            nc.vector.reciprocal(rcp[:], psDC[:])
            # normalized routing weights as rows [2, TC]
            nc.vector.tensor_scalar(crows[:], ev_rows[:], rcp[:], None, AL.mult)

            # broadcast slot rows along partitions via K=2 matmul with a
            # row-selector constant (sel2[:, j*128:(j+1)*128] has row j = 1)
            for j in range(2):
                psB = pspool.tile([128, TC], f32, tag="ps_small", name="psB")
                nc.tensor.matmul(psB[:], sel2_sb[:, j * 128:(j + 1) * 128],
                                 crows[:], start=True, stop=True)
                nc.vector.tensor_copy(cb[:, j * TC:(j + 1) * TC], psB[:])
                psM = pspool.tile([128, TC], f32, tag="ps_small", name="psM")
                nc.tensor.matmul(psM[:], sel2_sb[:, j * 128:(j + 1) * 128],
                                 s_rows[:], start=True, stop=True)
                nc.vector.tensor_scalar(Mj[:, j * TC:(j + 1) * TC], psM[:],
                                        eid_sb[:], None, AL.is_equal)

            # ---- phase 3: stacked A-projections ----
            psUA = pspool.tile([128, TC], f32, tag="psUG", bufs=2, name="psUA")
            for k in range(KT):
                nc.tensor.matmul(psUA[:],
                                 mm(A_sb[:, k * 2 * ER: k * 2 * ER + ER]),
                                 mm(xT_sb[:, k * TC:(k + 1) * TC]),
                                 start=(k == 0), stop=(k == KT - 1))
            nc.vector.tensor_copy(UA[:], psUA[:])
            psGA = pspool.tile([128, TC], f32, tag="psUG", bufs=2, name="psGA")
            for k in range(KT):
                nc.tensor.matmul(psGA[:],
                                 mm(A_sb[:, k * 2 * ER + ER:(k + 1) * 2 * ER]),
                                 mm(xT_sb[:, k * TC:(k + 1) * TC]),
                                 start=(k == 0), stop=(k == KT - 1))
            nc.vector.tensor_copy(GA[:], psGA[:])
            for j in range(2):
                nc.vector.tensor_tensor(mo(Ut[:, j * TC:(j + 1) * TC]), UA[:],
                                        Mj[:, j * TC:(j + 1) * TC], AL.mult)
                nc.vector.tensor_tensor(mo(Gt[:, j * TC:(j + 1) * TC]), GA[:],
                                        Mj[:, j * TC:(j + 1) * TC], AL.mult)

            # ---- phases 2+5+6: h-tile loop ----
            psV = pspool.tile([128, 2 * TC], f32, tag="psV", name="psV")
            KH = KT // 2 * 128          # half of the k columns (1024)
            QH = HT // 4 * 128          # quarter of the h columns (1408)
            wd_pre = {}                 # (di, q) -> prefetched tile
            pend_v = None               # delayed psV matmul (dA_t, ch_pair)

            def load_wd(di, q):
                t = wpool.tile([128, QH], SD, tag="wd", bufs=6, name="wd_t")
                nc.sync.dma_start(
                    out=mo(t[:]), in_=mo(d_wd[di][:, q * QH:(q + 1) * QH]))
                return t

            for i in range(HT):
                if 4 <= i < 10:
                    k6 = i - 4          # prefetch 6 wd quarters mid-loop
                    wd_pre[(k6 // 4, k6 % 4)] = load_wd(k6 // 4, k6 % 4)
                wu_h = []
                wg_h = []
                for hf in range(2):
                    wu_t = wpool.tile([128, KH], WUG, tag="wu", bufs=4)
                    nc.sync.dma_start(
                        out=mug(wu_t[:]),
                        in_=mug(d_wu[i][:, hf * KH:(hf + 1) * KH]))
                    wu_h.append(wu_t)
                    wg_t = wpool.tile([128, KH], WUG, tag="wg", bufs=4)
                    nc.sync.dma_start(
                        out=mug(wg_t[:]),
                        in_=mug(d_wg[i][:, hf * KH:(hf + 1) * KH]))
                    wg_h.append(wg_t)
                uB_t = bpool.tile([128, 128], SD, tag="uB")
                nc.sync.dma_start(out=mo(uB_t[:]), in_=mo(d_uB[i]))
                gB_t = bpool.tile([128, 128], SD, tag="gB")
                nc.sync.dma_start(out=mo(gB_t[:]), in_=mo(d_gB[i]))
                dA_t = bpool.tile([128, ER], SD, tag="dA")
                nc.sync.dma_start(out=mo(dA_t[:]), in_=mo(d_dA[i]))

                psUG = pspool.tile([128, 2 * TC], f32, tag="psUG", bufs=2,
                                   name="psUG")
                for k in range(KT):
                    w = wu_h[k // 8][:, (k % 8) * 128:(k % 8 + 1) * 128]
                    nc.tensor.matmul(psUG[:, 0:TC], mug(w),
                                     mug(xTb[:, k * TC:(k + 1) * TC]),
                                     start=(k == 0), stop=(k == KT - 1))
                for k in range(KT):
                    w = wg_h[k // 8][:, (k % 8) * 128:(k % 8 + 1) * 128]
                    nc.tensor.matmul(psUG[:, TC:2 * TC], mug(w),
                                     mug(xTb[:, k * TC:(k + 1) * TC]),
                                     start=(k == 0), stop=(k == KT - 1))
                if pend_v is not None:
                    pv_dA, pv_ch = pend_v
                    nc.tensor.matmul(psV[:], mm(pv_dA[:]), mm(pv_ch[:]),
                                     start=(i == 1), stop=False,
                                     skip_group_check=True)
                U_sb = spool.tile([128, TC], SD, tag="U_sb")
                nc.scalar.copy(U_sb[:], psUG[:, 0:TC])
                G_sb = spool.tile([128, TC], SD, tag="G_sb")
                nc.scalar.copy(G_sb[:], psUG[:, TC:2 * TC])

                psLO = pspool.tile([128, 4 * TC], f32, tag="psLO", bufs=2,
                                   name="psLO")
                # both slots' c*h in ONE tile so the down_A contraction is a
                # single [128,512] matmul per h-tile (avoids the whole-bank
                # has_written clear from a second start=True in the same bank)
                ch_pair = spool.tile([128, 2 * TC], SD, tag="chp", bufs=3)
                for j in range(2):
                    nc.tensor.matmul(psLO[:, (2 * j) * TC:(2 * j + 1) * TC],
                                     mm(uB_t[:]),
                                     mm(Ut[:, j * TC:(j + 1) * TC]),
                                     start=True, stop=True)
                    nc.tensor.matmul(psLO[:, (2 * j + 1) * TC:(2 * j + 2) * TC],
                                     mm(gB_t[:]),
                                     mm(Gt[:, j * TC:(j + 1) * TC]),
                                     start=True, stop=True)
                    tu = spool.tile([128, TC], SD, tag="tu")
                    nc.vector.tensor_tensor(
                        tu[:], U_sb[:], psLO[:, (2 * j) * TC:(2 * j + 1) * TC],
                        AL.add)
                    su = spool.tile([128, TC], SD, tag="su")
                    nc.scalar.activation(su[:], tu[:], AF.Silu)
                    tg = spool.tile([128, TC], SD, tag="tg")
                    nc.vector.tensor_tensor(
                        tg[:], G_sb[:],
                        psLO[:, (2 * j + 1) * TC:(2 * j + 2) * TC], AL.add)
                    hh = spool.tile([128, TC], SD, tag="hh")
                    nc.vector.tensor_tensor(hh[:], su[:], tg[:], AL.mult)
                    nc.vector.tensor_tensor(mo(ch_pair[:, j * TC:(j + 1) * TC]),
                                            hh[:],
                                            cb[:, j * TC:(j + 1) * TC], AL.mult)
                # psV matmul for tile i-1 is emitted AFTER tile i's base
                # matmuls: keeps the in-order PE queue from stalling on the
                # DVE chain that produces ch_pair (head-of-line blocking)
                nc.vector.tensor_tensor(mo(mixed[:, i * TC:(i + 1) * TC]),
                                        ch_pair[:, 0:TC], ch_pair[:, TC:2 * TC],
                                        AL.add)
                pend_v = (dA_t, ch_pair)

            pv_dA, pv_ch = pend_v
            nc.tensor.matmul(psV[:], mm(pv_dA[:]), mm(pv_ch[:]),
                             start=False, stop=True, skip_group_check=True)
            # masked v
            for j in range(2):
                nc.vector.tensor_tensor(mo(vt[:, j * TC:(j + 1) * TC]),
                                        psV[:, j * TC:(j + 1) * TC],
                                        Mj[:, j * TC:(j + 1) * TC], AL.mult)

            if DEBUG_TAPS:
                for nm, tl in [("crows", crows), ("srows", s_rows),
                               ("cb", cb), ("Mj", Mj), ("UA", UA),
                               ("GA", GA), ("vt", vt),
                               ("mixed0", mixed[:, 0:TC]),
                               ("mixed7", mixed[:, 7 * TC:8 * TC])]:
                    shp = [tl.shape[0], tl.shape[-1]]
                    dbg = nc.dram_tensor(f"dbg_{nm}", shp, f32,
                                         kind="ExternalOutput").ap()
                    nc.sync.dma_start(out=dbg[:], in_=tl[:])

            # ---- phase 7: down GEMM + LoRA-down ----
            for di in range(DT):
                wd_q = [wd_pre.get((di, q)) or load_wd(di, q)
                        for q in range(4)]
                psO = pspool.tile([128, TC], f32, tag="psUG", bufs=2, name="psO")
                for hk in range(HT):
                    w = wd_q[hk // 11][:, (hk % 11) * 128:(hk % 11 + 1) * 128]
                    nc.tensor.matmul(psO[:], mm(w),
                                     mm(mixed[:, hk * TC:(hk + 1) * TC]),
                                     start=(hk == 0), stop=False,
                                     skip_group_check=True)
                nc.tensor.matmul(psO[:], mm(dB_sb[:, di * 128:(di + 1) * 128]),
                                 mm(vt[:, 0:TC]), start=False, stop=False,
                                 skip_group_check=True)
                nc.tensor.matmul(psO[:], mm(dB_sb[:, di * 128:(di + 1) * 128]),
                                 mm(vt[:, TC:2 * TC]), start=False, stop=True,
                                 skip_group_check=True)
                o_sb = spool.tile([128, TC], f32, tag="o_sb")
                nc.scalar.copy(o_sb[:], psO[:])
                nc.sync.dma_start(out=d_out[di * 128:(di + 1) * 128, :],
                                  in_=o_sb[:])

    nc.compile()
    return nc


def _prep_shared(inputs):
    """Host-side layout prep of weight tensors (shared across cores)."""
    import ml_dtypes
    sd = _np_sd()
    sd_ug = (np.dtype(ml_dtypes.bfloat16) if MM_MODE in ("bf16", "hyb")
             else np.dtype(np.float32))
    f32 = np.float32

    def c(a, dt):
        return np.ascontiguousarray(a.astype(dt, copy=False))

    w_up, w_gate, w_down = inputs["w_up"], inputs["w_gate"], inputs["w_down"]
    wu = c(w_up.reshape(HT, 128, KT, 128).transpose(0, 3, 2, 1)
           .reshape(HT, 128, KT * 128), sd_ug)
    wg = c(w_gate.reshape(HT, 128, KT, 128).transpose(0, 3, 2, 1)
           .reshape(HT, 128, KT * 128), sd_ug)
    wd = c(w_down.reshape(DT, 128, HT, 128).transpose(0, 3, 2, 1)
           .reshape(DT, 128, HT * 128), sd)

    A_stack = np.concatenate([
        inputs["up_A"].reshape(ER, D),
        inputs["gate_A"].reshape(ER, D)], axis=0)          # [2*ER, D]
    # Ah[p, k*2ER + m] = A_stack[m, k*128+p]
    Ah = c(A_stack.reshape(2 * ER, KT, 128).transpose(2, 1, 0)
           .reshape(128, KT * 2 * ER), sd)

    up_B_all = (inputs["up_B"].transpose(0, 2, 1).reshape(ER, H)
                * ALPHA).astype(f32)
    gate_B_all = (inputs["gate_B"].transpose(0, 2, 1).reshape(ER, H)
                  * ALPHA).astype(f32)
    uB = c(up_B_all.reshape(ER, HT, 128).transpose(1, 0, 2), sd)
    gB = c(gate_B_all.reshape(ER, HT, 128).transpose(1, 0, 2), sd)

    down_A_all = inputs["down_A"].reshape(ER, H).astype(f32)
    dA = c(down_A_all.T.reshape(HT, 128, ER), sd)
    down_B_all = (inputs["down_B"].transpose(0, 2, 1).reshape(ER, D)
                  * ALPHA).astype(f32)
    dB = c(down_B_all, sd)

    gate_wT = inputs["gate_w"].T.astype(f32)               # [D, E]
    gw = c(gate_wT.reshape(KT, 128, E).transpose(1, 0, 2)
           .reshape(128, KT * E), f32)

    eid = (8.0 - (np.arange(128) // R)).astype(f32).reshape(128, 1)
    i8m = np.tile((8.0 - np.arange(E)).astype(f32), (128, 1))
    sel2 = np.zeros((2, 256), f32)
    sel2[0, 0:128] = 1.0
    sel2[1, 128:256] = 1.0

    return dict(wu=wu, wg=wg, wd=wd, Ah=Ah, uB=uB, gB=gB, dA=dA, dB=dB,
                gw=gw, eid=eid, i8m=i8m, sel2=sel2)


def kernel(**inputs):
    from concourse.bass_utils import run_bass_kernel_spmd

    inputs = {k: np.asarray(v) for k, v in inputs.items()}
    if "nc" not in _cache:
        _cache["nc"] = _build()
    nc = _cache["nc"]

    shared = _prep_shared(inputs)
    sd = _np_sd()
    x = inputs["x"].astype(np.float32)
    xt = x.reshape(T, D)

    in_maps = []
    for cix in range(NCORES):
        xc = xt[cix * TC:(cix + 1) * TC]                   # [TC, D]
        xT = np.ascontiguousarray(xc.T)                    # [D, TC] f32
        b = (cix * TC) // S
        bsr = np.zeros((1, 2), np.float32); bsr[0, b] = 1.0
        bsc = np.zeros((2, 1), np.float32); bsc[b, 0] = 1.0
        m = dict(shared)
        m["xT"] = xT.astype(sd) if MM_MODE == "bf16" else xT
        if MM_MODE == "bf16":
            m["xTr"] = xT
        m["bsr"] = bsr
        m["bsc"] = bsc
        in_maps.append(m)

    res = run_bass_kernel_spmd(nc, in_maps, list(range(NCORES)))
    out = np.empty((T, D), np.float32)
    for cix in range(NCORES):
        out[cix * TC:(cix + 1) * TC, :] = res.results[cix]["outT"].T
    return out.reshape(B, S, D)



# revision 2
# speedup vs baseline: 1.1871x; 1.1871x over previous
"""Trainium2 Bass kernel for nn_MistralMoLoraLayer (MoE-routed LoRA FFN).

Strategy: data-parallel over tokens (8 cores x 256 tokens), base FFN weights
replicated, all-expert LoRA replicated. The per-(batch,slot) softmax over the
sequence axis needs global denominators -> tiny [2,8] AllReduce.

Per-core math (all tiles [h/er/d partitions, tokens free]):
  router: logits = x @ gate_w.T (f32); top-2 (value,index) per token; exp; AR
          of per-batch-slot sums; weights w_j = exp_j / denom[batch, j]
  A-proj: UA/GA [E*R=128, t] = stacked up_A/gate_A @ x.T   (one K=128 chain)
  slot-mask trick: Ut_j = UA * M_j where M_j[e*R+r, t] = (sel_j(t)==e);
          lo_up_j[h,t] = (stacked up_B) @ Ut_j  == up_B[sel_j(t)] @ u_{sel_j(t)}
  h_j = silu(U + lo_up_j) * (G + lo_gate_j); ch_j = c_j * h_j
  mixed = ch_0 + ch_1
  v_j[er,t] = (stacked down_A) @ ch_j  (accumulated over h), masked by M_j
  outT[d,t] = w_down-chain @ mixed + (stacked down_B) @ v_0 + ... @ v_1

Everything off the router path runs in bf16 (weights streamed bf16, PSUM
accumulation f32); the router logits stay f32 so near-tie top-2 selections
match the reference.
"""

import numpy as np

# problem constants (hardcoded; kernel.py must be self-contained)
B, S, D, H, E, R, TOPK = 2, 1024, 2048, 5632, 8, 16, 2
ALPHA = 2.0
T = B * S
NCORES = 8
TC = T // NCORES           # 256 tokens per core
KT = D // 128              # 16 k-tiles over D
HT = H // 128              # 44 h-tiles
DT = D // 128              # 16 d-tiles
ER = E * R                 # 128

DEBUG_TAPS = False         # add intermediate-tensor outputs for debugging
SKIP_AR = False            # replace AllReduce with local copy (for TimelineSim)
WD_PRE = 12                # wd quarters prefetched during the h-loop

_cache = {}


def _build():
    import concourse.bacc as bacc
    import concourse.bass as bass
    import concourse.mybir as mybir
    import concourse.tile as tile
    from concourse.masks import make_identity

    f32 = mybir.dt.float32
    bf16 = mybir.dt.bfloat16
    AL = mybir.AluOpType
    AF = mybir.ActivationFunctionType

    nc = bacc.Bacc("TRN2", target_bir_lowering=False, debug=False,
                   num_devices=NCORES)

    # ---- DRAM I/O ----
    d_xT = nc.dram_tensor("xT", [D, TC], f32, kind="ExternalInput").ap()
    d_xTb = nc.dram_tensor("xTb", [D, TC], bf16, kind="ExternalInput").ap()
    d_gw = nc.dram_tensor("gw", [128, KT * E], f32, kind="ExternalInput").ap()
    d_wu = nc.dram_tensor("wu", [HT, 128, KT * 128], bf16, kind="ExternalInput").ap()
    d_wg = nc.dram_tensor("wg", [HT, 128, KT * 128], bf16, kind="ExternalInput").ap()
    d_wd = nc.dram_tensor("wd", [DT, 128, HT * 128], bf16, kind="ExternalInput").ap()
    d_A = nc.dram_tensor("Ah", [128, KT * 2 * ER], bf16, kind="ExternalInput").ap()
    d_uB = nc.dram_tensor("uB", [HT, 128, 128], bf16, kind="ExternalInput").ap()
    d_gB = nc.dram_tensor("gB", [HT, 128, 128], bf16, kind="ExternalInput").ap()
    d_dA = nc.dram_tensor("dA", [HT, 128, ER], bf16, kind="ExternalInput").ap()
    d_dB = nc.dram_tensor("dB", [128, D], bf16, kind="ExternalInput").ap()
    d_eid = nc.dram_tensor("eid", [128, 1], f32, kind="ExternalInput").ap()
    d_i8m = nc.dram_tensor("i8m", [128, E], f32, kind="ExternalInput").ap()
    d_bsr = nc.dram_tensor("bsr", [1, 2], f32, kind="ExternalInput").ap()
    d_bsc = nc.dram_tensor("bsc", [2, 1], f32, kind="ExternalInput").ap()
    d_sel2 = nc.dram_tensor("sel2", [2, 256], f32, kind="ExternalInput").ap()
    d_out = nc.dram_tensor("outT", [D, TC], f32, kind="ExternalOutput").ap()

    with tile.TileContext(nc) as tc:
        import contextlib
        ctx = contextlib.ExitStack()
        with ctx:
            cpool = ctx.enter_context(tc.tile_pool(name="const", bufs=1))
            wpool = ctx.enter_context(tc.tile_pool(name="wstream", bufs=2))
            bpool = ctx.enter_context(tc.tile_pool(name="bstream", bufs=3))
            spool = ctx.enter_context(tc.tile_pool(name="work", bufs=2))
            pspool = ctx.enter_context(
                tc.tile_pool(name="ps", bufs=1, space="PSUM"))
            drpool = ctx.enter_context(
                tc.tile_pool(name="dram", bufs=1, space="DRAM"))

            # ---- constants: tiny tiles FIRST so the router isn't queued
            # behind megabyte transfers on the HWDGE queue ----
            gw_sb = cpool.tile([128, KT * E], f32, name="gw_sb")
            nc.sync.dma_start(out=gw_sb[:], in_=d_gw[:])
            eid_sb = cpool.tile([128, 1], f32, name="eid_sb")
            nc.sync.dma_start(out=eid_sb[:], in_=d_eid[:])
            i8m_sb = cpool.tile([128, E], f32, name="i8m_sb")
            nc.sync.dma_start(out=i8m_sb[:], in_=d_i8m[:])
            bsr_sb = cpool.tile([1, 2], f32, name="bsr_sb")
            nc.sync.dma_start(out=bsr_sb[:], in_=d_bsr[:])
            bsc_sb = cpool.tile([2, 1], f32, name="bsc_sb")
            nc.sync.dma_start(out=bsc_sb[:], in_=d_bsc[:])
            sel2_sb = cpool.tile([2, 256], f32, name="sel2_sb")
            nc.sync.dma_start(out=sel2_sb[:], in_=d_sel2[:])

            # router x (f32), then bf16 x, then LoRA stacks
            xT_sb = cpool.tile([128, KT * TC], f32, name="xT_sb")
            for k in range(KT):
                nc.sync.dma_start(out=xT_sb[:, k * TC:(k + 1) * TC],
                                  in_=d_xT[k * 128:(k + 1) * 128, :])
            xTb = cpool.tile([128, KT * TC], bf16, name="xTb")
            for k in range(KT):
                nc.sync.dma_start(out=xTb[:, k * TC:(k + 1) * TC],
                                  in_=d_xTb[k * 128:(k + 1) * 128, :])
            A_sb = cpool.tile([128, KT * 2 * ER], bf16, name="A_sb")
            nc.sync.dma_start(out=A_sb[:], in_=d_A[:])
            dB_sb = cpool.tile([128, D], bf16, name="dB_sb")
            nc.sync.dma_start(out=dB_sb[:], in_=d_dB[:])

            ident = cpool.tile([128, 128], f32, name="ident")
            make_identity(nc, ident)
            ones_col = cpool.tile([128, 1], f32, name="ones_col")
            nc.vector.memset(ones_col, 1.0)

            mixed = cpool.tile([128, HT * TC], bf16, name="mixed")
            ev_rows = cpool.tile([2, TC], f32, name="ev_rows")
            s_rows = cpool.tile([2, TC], f32, name="s_rows")
            crows = cpool.tile([2, TC], f32, name="crows")
            cb = cpool.tile([128, 2 * TC], bf16, name="cb")
            Mj = cpool.tile([128, 2 * TC], f32, name="Mj")
            UA = cpool.tile([128, TC], f32, name="UA")
            GA = cpool.tile([128, TC], f32, name="GA")
            Ut = cpool.tile([128, 2 * TC], bf16, name="Ut")
            Gt = cpool.tile([128, 2 * TC], bf16, name="Gt")
            vt = cpool.tile([128, 2 * TC], bf16, name="vt")

            # ---- phase 1: router (f32) ----
            den_parts = cpool.tile([1, 4], f32, name="den_parts")
            for tt in range(2):
                psL = pspool.tile([128, TC], f32, tag="ps_small", name="psL")
                for k in range(KT):
                    nc.tensor.matmul(
                        psL[:, 0:E],
                        xT_sb[:, k * TC + tt * 128: k * TC + tt * 128 + 128],
                        gw_sb[:, k * E:(k + 1) * E],
                        start=(k == 0), stop=(k == KT - 1))
                L = spool.tile([128, E], f32, tag="L")
                nc.vector.tensor_copy(L[:], psL[:, 0:E])
                mx1 = spool.tile([128, 1], f32, tag="mx1")
                nc.vector.tensor_reduce(mx1[:], L[:], mybir.AxisListType.X, AL.max)
                msk = spool.tile([128, E], f32, tag="msk")
                nc.vector.tensor_scalar(msk[:], L[:], mx1[:], None, AL.is_equal)
                mi = spool.tile([128, E], f32, tag="mi")
                nc.vector.tensor_tensor(mi[:], msk[:], i8m_sb[:], AL.mult)
                svals = spool.tile([128, 2], f32, tag="svals")
                nc.vector.tensor_reduce(svals[:, 0:1], mi[:],
                                        mybir.AxisListType.X, AL.max)
                evals = spool.tile([128, 2], f32, tag="evals")
                nc.scalar.activation(evals[:, 0:1], mx1[:], AF.Exp)
                # mask out slot-0 winner, find second max
                big = spool.tile([128, E], f32, tag="big")
                nc.vector.tensor_scalar(big[:], msk[:], 1e30, None, AL.mult)
                L2 = spool.tile([128, E], f32, tag="L2")
                nc.vector.tensor_tensor(L2[:], L[:], big[:], AL.subtract)
                mx2 = spool.tile([128, 1], f32, tag="mx2")
                nc.vector.tensor_reduce(mx2[:], L2[:], mybir.AxisListType.X, AL.max)
                msk2 = spool.tile([128, E], f32, tag="msk2")
                nc.vector.tensor_scalar(msk2[:], L2[:], mx2[:], None, AL.is_equal)
                mi2 = spool.tile([128, E], f32, tag="mi2")
                nc.vector.tensor_tensor(mi2[:], msk2[:], i8m_sb[:], AL.mult)
                nc.vector.tensor_reduce(svals[:, 1:2], mi2[:],
                                        mybir.AxisListType.X, AL.max)
                nc.scalar.activation(evals[:, 1:2], mx2[:], AF.Exp)
                # per-tile partial denominators: [1,2] = ones.T @ evals
                psd = pspool.tile([1, 2], f32, tag="ps_small", name="psd")
                nc.tensor.matmul(psd[:], ones_col[:], evals[:],
                                 start=True, stop=True)
                nc.vector.tensor_copy(den_parts[:, tt * 2:(tt + 1) * 2], psd[:])
                # transpose evals/svals -> rows
                psT = pspool.tile([2, 128], f32, tag="ps_small", name="psT")
                nc.tensor.transpose(psT[:], evals[:], ident[:])
                nc.vector.tensor_copy(ev_rows[:, tt * 128:(tt + 1) * 128], psT[:])
                psT2 = pspool.tile([2, 128], f32, tag="ps_small", name="psT2")
                nc.tensor.transpose(psT2[:], svals[:], ident[:])
                nc.vector.tensor_copy(s_rows[:, tt * 128:(tt + 1) * 128], psT2[:])

            # combine partials, AllReduce [2,8] (row b = batch, cols 0:2 used)
            drow = cpool.tile([1, 2], f32, name="drow")
            nc.vector.tensor_tensor(drow[:], den_parts[:, 0:2],
                                    den_parts[:, 2:4], AL.add)
            ar_sb = cpool.tile([2, 8], f32, name="ar_sb")
            nc.vector.memset(ar_sb, 0.0)
            psAR = pspool.tile([2, 2], f32, tag="ps_small", name="psAR")
            nc.tensor.matmul(psAR[:], bsr_sb[:], drow[:], start=True, stop=True)
            nc.vector.tensor_copy(ar_sb[:, 0:2], psAR[:])
            ar_in = drpool.tile([2, 8], f32, name="ar_in")
            ar_out = drpool.tile([2, 8], f32, name="ar_out", addr_space="Shared")
            nc.gpsimd.dma_start(out=ar_in[:], in_=ar_sb[:])
            if SKIP_AR:
                nc.gpsimd.dma_start(out=ar_out[:], in_=ar_in[:])
            else:
                nc.gpsimd.collective_compute(
                    "AllReduce", AL.add,
                    replica_groups=[list(range(NCORES))],
                    ins=[ar_in.opt()], outs=[ar_out.opt()])
            den2 = cpool.tile([2, 8], f32, name="den2")
            nc.gpsimd.dma_start(out=den2[:], in_=ar_out[:])
            # select this core's batch row -> [2(slots),1], reciprocal
            psDC = pspool.tile([2, 1], f32, tag="ps_small", name="psDC")
            nc.tensor.matmul(psDC[:], den2[:, 0:2], bsc_sb[:],
                             start=True, stop=True)
            rcp = cpool.tile([2, 1], f32, name="rcp")
            nc.vector.reciprocal(rcp[:], psDC[:])
            # normalized routing weights as rows [2, TC]
            nc.vector.tensor_scalar(crows[:], ev_rows[:], rcp[:], None, AL.mult)

            # broadcast slot rows along partitions via K=2 matmul with a
            # row-selector constant (sel2[:, j*128:(j+1)*128] has row j = 1)
            for j in range(2):
                psB = pspool.tile([128, TC], f32, tag="ps_small", name="psB")
                nc.tensor.matmul(psB[:], sel2_sb[:, j * 128:(j + 1) * 128],
                                 crows[:], start=True, stop=True)
                nc.vector.tensor_copy(cb[:, j * TC:(j + 1) * TC], psB[:])
                psM = pspool.tile([128, TC], f32, tag="ps_small", name="psM")
                nc.tensor.matmul(psM[:], sel2_sb[:, j * 128:(j + 1) * 128],
                                 s_rows[:], start=True, stop=True)
                nc.vector.tensor_scalar(Mj[:, j * TC:(j + 1) * TC], psM[:],
                                        eid_sb[:], None, AL.is_equal)

            # ---- phase 3: stacked A-projections (bf16) ----
            psUA = pspool.tile([128, TC], f32, tag="psUG", bufs=2, name="psUA")
            for k in range(KT):
                nc.tensor.matmul(psUA[:],
                                 A_sb[:, k * 2 * ER: k * 2 * ER + ER],
                                 xTb[:, k * TC:(k + 1) * TC],
                                 start=(k == 0), stop=(k == KT - 1))
            nc.vector.tensor_copy(UA[:], psUA[:])
            psGA = pspool.tile([128, TC], f32, tag="psUG", bufs=2, name="psGA")
            for k in range(KT):
                nc.tensor.matmul(psGA[:],
                                 A_sb[:, k * 2 * ER + ER:(k + 1) * 2 * ER],
                                 xTb[:, k * TC:(k + 1) * TC],
                                 start=(k == 0), stop=(k == KT - 1))
            nc.vector.tensor_copy(GA[:], psGA[:])
            for j in range(2):
                nc.vector.tensor_tensor(Ut[:, j * TC:(j + 1) * TC], UA[:],
                                        Mj[:, j * TC:(j + 1) * TC], AL.mult)
                nc.vector.tensor_tensor(Gt[:, j * TC:(j + 1) * TC], GA[:],
                                        Mj[:, j * TC:(j + 1) * TC], AL.mult)

            # ---- phases 2+5+6: h-tile loop ----
            psV = pspool.tile([128, 2 * TC], f32, tag="psV", name="psV")
            KH = KT // 2 * 128          # half of the k columns (1024)
            QH = HT // 4 * 128          # quarter of the h columns (1408)
            wd_pre = {}                 # (di, q) -> prefetched tile
            pend_v = None               # delayed psV matmul (dA_t, ch_pair)

            def load_wd(di, q):
                t = wpool.tile([128, QH], bf16, tag="wd", bufs=WD_PRE + 4,
                               name="wd_t")
                nc.sync.dma_start(
                    out=t[:], in_=d_wd[di][:, q * QH:(q + 1) * QH])
                return t

            for i in range(HT):
                if 4 <= i < 4 + WD_PRE:
                    k6 = i - 4          # prefetch wd quarters mid-loop
                    wd_pre[(k6 // 4, k6 % 4)] = load_wd(k6 // 4, k6 % 4)
                wu_h = []
                wg_h = []
                for hf in range(2):
                    wu_t = wpool.tile([128, KH], bf16, tag="wu", bufs=4)
                    nc.sync.dma_start(
                        out=wu_t[:],
                        in_=d_wu[i][:, hf * KH:(hf + 1) * KH])
                    wu_h.append(wu_t)
                    wg_t = wpool.tile([128, KH], bf16, tag="wg", bufs=4)
                    nc.sync.dma_start(
                        out=wg_t[:],
                        in_=d_wg[i][:, hf * KH:(hf + 1) * KH])
                    wg_h.append(wg_t)
                uB_t = bpool.tile([128, 128], bf16, tag="uB")
                nc.sync.dma_start(out=uB_t[:], in_=d_uB[i])
                gB_t = bpool.tile([128, 128], bf16, tag="gB")
                nc.sync.dma_start(out=gB_t[:], in_=d_gB[i])
                dA_t = bpool.tile([128, ER], bf16, tag="dA")
                nc.sync.dma_start(out=dA_t[:], in_=d_dA[i])

                psUG = pspool.tile([128, 2 * TC], f32, tag="psUG", bufs=2,
                                   name="psUG")
                for k in range(KT):
                    w = wu_h[k // 8][:, (k % 8) * 128:(k % 8 + 1) * 128]
                    nc.tensor.matmul(psUG[:, 0:TC], w,
                                     xTb[:, k * TC:(k + 1) * TC],
                                     start=(k == 0), stop=(k == KT - 1))
                for k in range(KT):
                    w = wg_h[k // 8][:, (k % 8) * 128:(k % 8 + 1) * 128]
                    nc.tensor.matmul(psUG[:, TC:2 * TC], w,
                                     xTb[:, k * TC:(k + 1) * TC],
                                     start=(k == 0), stop=(k == KT - 1))
                if pend_v is not None:
                    pv_dA, pv_ch = pend_v
                    nc.tensor.matmul(psV[:], pv_dA[:], pv_ch[:],
                                     start=(i == 1), stop=False,
                                     skip_group_check=True)
                U_sb = spool.tile([128, TC], f32, tag="U_sb")
                nc.scalar.copy(U_sb[:], psUG[:, 0:TC])
                G_sb = spool.tile([128, TC], f32, tag="G_sb")
                nc.scalar.copy(G_sb[:], psUG[:, TC:2 * TC])

                psLO = pspool.tile([128, 4 * TC], f32, tag="psLO", bufs=2,
                                   name="psLO")
                # all four B-proj matmuls are emitted BEFORE any DVE consumer
                # of psLO: the tile-granular WAR tracking otherwise stalls the
                # in-order PE queue on slot-0's DVE reads (663ns/iter)
                for j in range(2):
                    nc.tensor.matmul(psLO[:, (2 * j) * TC:(2 * j + 1) * TC],
                                     uB_t[:],
                                     Ut[:, j * TC:(j + 1) * TC],
                                     start=True, stop=True)
                    nc.tensor.matmul(psLO[:, (2 * j + 1) * TC:(2 * j + 2) * TC],
                                     gB_t[:],
                                     Gt[:, j * TC:(j + 1) * TC],
                                     start=True, stop=True)
                # both slots' c*h in ONE tile so the down_A contraction is a
                # single [128,512] matmul per h-tile
                ch_pair = spool.tile([128, 2 * TC], bf16, tag="chp", bufs=3)
                for j in range(2):
                    tu = spool.tile([128, TC], bf16, tag="tu")
                    nc.vector.tensor_tensor(
                        tu[:], U_sb[:], psLO[:, (2 * j) * TC:(2 * j + 1) * TC],
                        AL.add)
                    su = spool.tile([128, TC], bf16, tag="su")
                    nc.scalar.activation(su[:], tu[:], AF.Silu)
                    tg = spool.tile([128, TC], bf16, tag="tg")
                    nc.vector.tensor_tensor(
                        tg[:], G_sb[:],
                        psLO[:, (2 * j + 1) * TC:(2 * j + 2) * TC], AL.add)
                    hh = spool.tile([128, TC], bf16, tag="hh")
                    nc.vector.tensor_tensor(hh[:], su[:], tg[:], AL.mult)
                    nc.vector.tensor_tensor(ch_pair[:, j * TC:(j + 1) * TC],
                                            hh[:],
                                            cb[:, j * TC:(j + 1) * TC], AL.mult)
                # psV matmul for tile i-1 is emitted AFTER tile i's base
                # matmuls: keeps the in-order PE queue from stalling on the
                # DVE chain that produces ch_pair (head-of-line blocking)
                nc.vector.tensor_tensor(mixed[:, i * TC:(i + 1) * TC],
                                        ch_pair[:, 0:TC], ch_pair[:, TC:2 * TC],
                                        AL.add)
                pend_v = (dA_t, ch_pair)

            pv_dA, pv_ch = pend_v
            nc.tensor.matmul(psV[:], pv_dA[:], pv_ch[:],
                             start=False, stop=True, skip_group_check=True)
            # masked v
            for j in range(2):
                nc.vector.tensor_tensor(vt[:, j * TC:(j + 1) * TC],
                                        psV[:, j * TC:(j + 1) * TC],
                                        Mj[:, j * TC:(j + 1) * TC], AL.mult)

            if DEBUG_TAPS:
                for nm, tl in [("crows", crows), ("srows", s_rows),
                               ("cb", cb), ("Mj", Mj), ("UA", UA),
                               ("GA", GA), ("vt", vt),
                               ("mixed0", mixed[:, 0:TC]),
                               ("mixed7", mixed[:, 7 * TC:8 * TC])]:
                    shp = [tl.shape[0], tl.shape[-1]]
                    dbg = nc.dram_tensor(f"dbg_{nm}", shp, f32,
                                         kind="ExternalOutput").ap()
                    nc.sync.dma_start(out=dbg[:], in_=tl[:])

            # ---- phase 7: down GEMM + LoRA-down ----
            for di in range(DT):
                wd_q = [wd_pre.get((di, q)) or load_wd(di, q)
                        for q in range(4)]
                psO = pspool.tile([128, TC], f32, tag="psUG", bufs=2, name="psO")
                for hk in range(HT):
                    w = wd_q[hk // 11][:, (hk % 11) * 128:(hk % 11 + 1) * 128]
                    nc.tensor.matmul(psO[:], w,
                                     mixed[:, hk * TC:(hk + 1) * TC],
                                     start=(hk == 0), stop=False,
                                     skip_group_check=True)
                nc.tensor.matmul(psO[:], dB_sb[:, di * 128:(di + 1) * 128],
                                 vt[:, 0:TC], start=False, stop=False,
                                 skip_group_check=True)
                nc.tensor.matmul(psO[:], dB_sb[:, di * 128:(di + 1) * 128],
                                 vt[:, TC:2 * TC], start=False, stop=True,
                                 skip_group_check=True)
                o_sb = spool.tile([128, TC], f32, tag="o_sb")
                nc.scalar.copy(o_sb[:], psO[:])
                nc.sync.dma_start(out=d_out[di * 128:(di + 1) * 128, :],
                                  in_=o_sb[:])

    nc.compile()
    return nc


def _prep_shared(inputs):
    """Host-side layout prep of weight tensors (shared across cores)."""
    import ml_dtypes
    bf16 = np.dtype(ml_dtypes.bfloat16)
    f32 = np.float32

    def c(a, dt):
        return np.ascontiguousarray(a.astype(dt, copy=False))

    w_up, w_gate, w_down = inputs["w_up"], inputs["w_gate"], inputs["w_down"]
    wu = c(w_up.reshape(HT, 128, KT, 128).transpose(0, 3, 2, 1)
           .reshape(HT, 128, KT * 128), bf16)
    wg = c(w_gate.reshape(HT, 128, KT, 128).transpose(0, 3, 2, 1)
           .reshape(HT, 128, KT * 128), bf16)
    wd = c(w_down.reshape(DT, 128, HT, 128).transpose(0, 3, 2, 1)
           .reshape(DT, 128, HT * 128), bf16)

    A_stack = np.concatenate([
        inputs["up_A"].reshape(ER, D),
        inputs["gate_A"].reshape(ER, D)], axis=0)          # [2*ER, D]
    # Ah[p, k*2ER + m] = A_stack[m, k*128+p]
    Ah = c(A_stack.reshape(2 * ER, KT, 128).transpose(2, 1, 0)
           .reshape(128, KT * 2 * ER), bf16)

    up_B_all = (inputs["up_B"].transpose(0, 2, 1).reshape(ER, H)
                * ALPHA).astype(f32)
    gate_B_all = (inputs["gate_B"].transpose(0, 2, 1).reshape(ER, H)
                  * ALPHA).astype(f32)
    uB = c(up_B_all.reshape(ER, HT, 128).transpose(1, 0, 2), bf16)
    gB = c(gate_B_all.reshape(ER, HT, 128).transpose(1, 0, 2), bf16)

    down_A_all = inputs["down_A"].reshape(ER, H).astype(f32)
    dA = c(down_A_all.T.reshape(HT, 128, ER), bf16)
    down_B_all = (inputs["down_B"].transpose(0, 2, 1).reshape(ER, D)
                  * ALPHA).astype(f32)
    dB = c(down_B_all, bf16)

    gate_wT = inputs["gate_w"].T.astype(f32)               # [D, E]
    gw = c(gate_wT.reshape(KT, 128, E).transpose(1, 0, 2)
           .reshape(128, KT * E), f32)

    eid = (8.0 - (np.arange(128) // R)).astype(f32).reshape(128, 1)
    i8m = np.tile((8.0 - np.arange(E)).astype(f32), (128, 1))
    sel2 = np.zeros((2, 256), f32)
    sel2[0, 0:128] = 1.0
    sel2[1, 128:256] = 1.0

    return dict(wu=wu, wg=wg, wd=wd, Ah=Ah, uB=uB, gB=gB, dA=dA, dB=dB,
                gw=gw, eid=eid, i8m=i8m, sel2=sel2)


def kernel(**inputs):
    import ml_dtypes
    from concourse.bass_utils import run_bass_kernel_spmd

    bf16 = np.dtype(ml_dtypes.bfloat16)
    inputs = {k: np.asarray(v) for k, v in inputs.items()}
    if "nc" not in _cache:
        _cache["nc"] = _build()
    nc = _cache["nc"]

    shared = _prep_shared(inputs)
    x = inputs["x"].astype(np.float32)
    xt = x.reshape(T, D)

    in_maps = []
    for cix in range(NCORES):
        xc = xt[cix * TC:(cix + 1) * TC]                   # [TC, D]
        xT = np.ascontiguousarray(xc.T)                    # [D, TC] f32
        b = (cix * TC) // S
        bsr = np.zeros((1, 2), np.float32); bsr[0, b] = 1.0
        bsc = np.zeros((2, 1), np.float32); bsc[b, 0] = 1.0
        m = dict(shared)
        m["xT"] = xT
        m["xTb"] = np.ascontiguousarray(xT.astype(bf16))
        m["bsr"] = bsr
        m["bsc"] = bsc
        in_maps.append(m)

    res = run_bass_kernel_spmd(nc, in_maps, list(range(NCORES)))
    out = np.empty((T, D), np.float32)
    for cix in range(NCORES):
        out[cix * TC:(cix + 1) * TC, :] = res.results[cix]["outT"].T
    return out.reshape(B, S, D)


# revision 50
# speedup vs baseline: 1.3399x; 1.1287x over previous
"""Trainium2 Bass kernel for nn_MistralMoLoraLayer (MoE-routed LoRA FFN).

Strategy: data-parallel over tokens (8 cores x 256 tokens), base FFN weights
replicated, all-expert LoRA replicated. The per-(batch,slot) softmax over the
sequence axis needs global denominators -> tiny [2,8] AllReduce.

Per-core math (all tiles [h/er/d partitions, tokens free]):
  router: logits = x @ gate_w.T (f32); top-2 (value,index) per token; exp; AR
          of per-batch-slot sums; weights w_j = exp_j / denom[batch, j]
  A-proj: UA/GA [E*R=128, t] = stacked up_A/gate_A @ x.T   (one K=128 chain)
  slot-mask trick: Ut_j = UA * M_j where M_j[e*R+r, t] = (sel_j(t)==e);
          lo_up_j[h,t] = (stacked up_B) @ Ut_j  == up_B[sel_j(t)] @ u_{sel_j(t)}
  h_j = silu(U + lo_up_j) * (G + lo_gate_j); ch_j = c_j * h_j
  mixed = ch_0 + ch_1
  v_j[er,t] = (stacked down_A) @ ch_j  (accumulated over h), masked by M_j
  outT[d,t] = w_down-chain @ mixed + (stacked down_B) @ v_0 + ... @ v_1

Everything off the router path runs in bf16 (weights streamed bf16, PSUM
accumulation f32); the router logits stay f32 so near-tie top-2 selections
match the reference.
"""

import numpy as np

# problem constants (hardcoded; kernel.py must be self-contained)
B, S, D, H, E, R, TOPK = 2, 1024, 2048, 5632, 8, 16, 2
ALPHA = 2.0
T = B * S
NCORES = 8
TC = T // NCORES           # 256 tokens per core
KT = D // 128              # 16 k-tiles over D
HT = H // 128              # 44 h-tiles
DT = D // 128              # 16 d-tiles
ER = E * R                 # 128

DEBUG_TAPS = False         # add intermediate-tensor outputs for debugging
SKIP_AR = False            # replace AllReduce with local copy (for TimelineSim)
WD_PRE = 6                 # wd halves prefetched during the h-loop

_cache = {}


def _build():
    import concourse.bacc as bacc
    import concourse.bass as bass
    import concourse.mybir as mybir
    import concourse.tile as tile
    from concourse.masks import make_identity

    f32 = mybir.dt.float32
    bf16 = mybir.dt.bfloat16
    AL = mybir.AluOpType
    AF = mybir.ActivationFunctionType

    nc = bacc.Bacc("TRN2", target_bir_lowering=False, debug=False,
                   num_devices=NCORES)

    # ---- DRAM I/O (host-prepped layouts; single-DMA where possible) ----
    d_xT = nc.dram_tensor("xT", [128, KT * TC], f32, kind="ExternalInput").ap()
    d_xTb = nc.dram_tensor("xTb", [128, KT * TC], bf16, kind="ExternalInput").ap()
    d_gw = nc.dram_tensor("gw", [128, KT * E], f32, kind="ExternalInput").ap()
    d_wu = nc.dram_tensor("wu", [HT, 128, KT * 128], bf16, kind="ExternalInput").ap()
    d_wg = nc.dram_tensor("wg", [HT, 128, KT * 128], bf16, kind="ExternalInput").ap()
    d_wd = nc.dram_tensor("wd", [DT, 128, HT * 128], bf16, kind="ExternalInput").ap()
    d_A = nc.dram_tensor("Ah", [128, KT * 2 * ER], bf16, kind="ExternalInput").ap()
    # per-h-tile LoRA pack: cols 0:128 up_B, 128:256 gate_B, 256:384 down_A
    d_Bp = nc.dram_tensor("Bp", [HT, 128, 384], bf16, kind="ExternalInput").ap()
    d_dB = nc.dram_tensor("dB", [128, D], bf16, kind="ExternalInput").ap()
    # col 0 = eid, cols 1:9 = i8m
    d_ei = nc.dram_tensor("ei", [128, 1 + E], f32, kind="ExternalInput").ap()
    # bscat[s, 2b+s] = 1 for this core's batch b; bmask = 1 at cols 2b,2b+1
    d_bscat = nc.dram_tensor("bscat", [2, 4], f32, kind="ExternalInput").ap()
    d_bmask = nc.dram_tensor("bmask", [128, 4], f32, kind="ExternalInput").ap()
    d_sel2 = nc.dram_tensor("sel2", [2, 256], f32, kind="ExternalInput").ap()
    d_out = nc.dram_tensor("outT", [D, TC], bf16, kind="ExternalOutput").ap()

    with tile.TileContext(nc) as tc:
        import contextlib
        ctx = contextlib.ExitStack()
        with ctx:
            cpool = ctx.enter_context(tc.tile_pool(name="const", bufs=1))
            wpool = ctx.enter_context(tc.tile_pool(name="wstream", bufs=2))
            bpool = ctx.enter_context(tc.tile_pool(name="bstream", bufs=3))
            spool = ctx.enter_context(tc.tile_pool(name="work", bufs=2))
            pspool = ctx.enter_context(
                tc.tile_pool(name="ps", bufs=1, space="PSUM"))
            drpool = ctx.enter_context(
                tc.tile_pool(name="dram", bufs=1, space="DRAM"))

            # ---- DMA plan: SP queue carries x + the wu/wg stream; the
            # Activation queue carries everything else (the SP sequencer
            # costs ~0.9us per dma_start, so queue assignment matters) ----
            xTb = cpool.tile([128, KT * TC], bf16, name="xTb")
            nc.sync.dma_start(out=xTb[:], in_=d_xTb[:])
            xT_sb = cpool.tile([128, KT * TC], f32, name="xT_sb")
            HKT = KT // 2
            for hx in range(2):
                nc.sync.dma_start(
                    out=xT_sb[:, hx * HKT * TC:(hx + 1) * HKT * TC],
                    in_=d_xT[:, hx * HKT * TC:(hx + 1) * HKT * TC])

            # warm both activation tables first thing on the Act engine (Silu
            # then Exp, so Exp is resident for the router; Silu reloads once
            # at h-loop start, off the AR critical path)
            warm = cpool.tile([1, 2], f32, name="warm")
            nc.vector.memset(warm, 0.0)
            nc.scalar.activation(warm[:, 0:1], warm[:, 0:1], AF.Silu)
            nc.scalar.activation(warm[:, 1:2], warm[:, 1:2], AF.Exp)

            gw_sb = cpool.tile([128, KT * E], f32, name="gw_sb")
            nc.scalar.dma_start(out=gw_sb[:], in_=d_gw[:])
            ei_sb = cpool.tile([128, 1 + E], f32, name="ei_sb")
            nc.scalar.dma_start(out=ei_sb[:], in_=d_ei[:])
            eid_sb = ei_sb[:, 0:1]
            i8m_sb = ei_sb[:, 1:1 + E]
            bscat_sb = cpool.tile([2, 4], f32, name="bscat_sb")
            nc.scalar.dma_start(out=bscat_sb[:], in_=d_bscat[:])
            bmask_sb = cpool.tile([128, 4], f32, name="bmask_sb")
            nc.scalar.dma_start(out=bmask_sb[:], in_=d_bmask[:])
            sel2_sb = cpool.tile([2, 256], f32, name="sel2_sb")
            nc.scalar.dma_start(out=sel2_sb[:], in_=d_sel2[:])
            A_sb = cpool.tile([128, KT * 2 * ER], bf16, name="A_sb")
            nc.scalar.dma_start(out=A_sb[:], in_=d_A[:])
            dB_sb = cpool.tile([128, D], bf16, name="dB_sb")
            nc.scalar.dma_start(out=dB_sb[:], in_=d_dB[:])

            ident = cpool.tile([128, 128], f32, name="ident")
            make_identity(nc, ident)
            ones_col = cpool.tile([128, 1], f32, name="ones_col")
            nc.vector.memset(ones_col, 1.0)
            ones_row = cpool.tile([1, 128], f32, name="ones_row")
            nc.vector.memset(ones_row, 1.0)
            ones2 = cpool.tile([2, 128], f32, name="ones2")
            nc.vector.memset(ones2, 1.0)

            mixed = cpool.tile([128, HT * TC], bf16, name="mixed")
            ev_rows = cpool.tile([2, TC], f32, name="ev_rows")
            s_rows = cpool.tile([2, TC], f32, name="s_rows")
            cbraw = cpool.tile([128, 2 * TC], f32, name="cbraw")
            cb = cpool.tile([128, 2 * TC], bf16, name="cb")
            Mj = cpool.tile([128, 2 * TC], f32, name="Mj")
            UA = cpool.tile([128, TC], f32, name="UA")
            GA = cpool.tile([128, TC], f32, name="GA")
            Ut = cpool.tile([128, 2 * TC], bf16, name="Ut")
            Gt = cpool.tile([128, 2 * TC], bf16, name="Gt")
            vt = cpool.tile([128, 2 * TC], bf16, name="vt")

            # ---- phase 0: stacked A-projections (xTb lands before the f32
            # router x, so these PE chains fill the front-end DMA wait) ----
            psUA = pspool.tile([128, TC], f32, tag="psV", name="psUA")
            for k in range(KT):
                nc.tensor.matmul(psUA[:],
                                 A_sb[:, k * 2 * ER: k * 2 * ER + ER],
                                 xTb[:, k * TC:(k + 1) * TC],
                                 start=(k == 0), stop=(k == KT - 1))
            nc.vector.tensor_copy(UA[:], psUA[:])
            psGA = pspool.tile([128, TC], f32, tag="psV", name="psGA")
            for k in range(KT):
                nc.tensor.matmul(psGA[:],
                                 A_sb[:, k * 2 * ER + ER:(k + 1) * 2 * ER],
                                 xTb[:, k * TC:(k + 1) * TC],
                                 start=(k == 0), stop=(k == KT - 1))
            nc.vector.tensor_copy(GA[:], psGA[:])

            # ---- phase 1a: router logit chains (PE) ----
            psL = {}
            for tt in range(2):
                psL[tt] = pspool.tile([128, TC], f32, tag="psUG", bufs=2,
                                      name=f"psL{tt}")
                for k in range(KT):
                    nc.tensor.matmul(
                        psL[tt][:, 0:E],
                        xT_sb[:, k * TC + tt * 128: k * TC + tt * 128 + 128],
                        gw_sb[:, k * E:(k + 1) * E],
                        start=(k == 0), stop=(k == KT - 1))

            # ---- phase 1b: top-2 select + exp (both token-tiles overlap) ----
            den_parts = cpool.tile([1, 4], f32, name="den_parts")
            evs, svs = {}, {}
            for tt in range(2):
                L = spool.tile([128, E], f32, tag="L")
                nc.vector.tensor_copy(L[:], psL[tt][:, 0:E])
                mx1 = spool.tile([128, 1], f32, tag="mx1")
                nc.vector.tensor_reduce(mx1[:], L[:], mybir.AxisListType.X, AL.max)
                msk = spool.tile([128, E], f32, tag="msk")
                nc.vector.tensor_scalar(msk[:], L[:], mx1[:], None, AL.is_equal)
                mi = spool.tile([128, E], f32, tag="mi")
                nc.vector.tensor_tensor(mi[:], msk[:], i8m_sb[:], AL.mult)
                svals = spool.tile([128, 2], f32, tag="svals")
                nc.vector.tensor_reduce(svals[:, 0:1], mi[:],
                                        mybir.AxisListType.X, AL.max)
                evals = spool.tile([128, 2], f32, tag="evals")
                nc.scalar.activation(evals[:, 0:1], mx1[:], AF.Exp)
                # mask out slot-0 winner, find second max
                big = spool.tile([128, E], f32, tag="big")
                nc.vector.tensor_scalar(big[:], msk[:], 1e30, None, AL.mult)
                L2 = spool.tile([128, E], f32, tag="L2")
                nc.vector.tensor_tensor(L2[:], L[:], big[:], AL.subtract)
                mx2 = spool.tile([128, 1], f32, tag="mx2")
                nc.vector.tensor_reduce(mx2[:], L2[:], mybir.AxisListType.X, AL.max)
                msk2 = spool.tile([128, E], f32, tag="msk2")
                nc.vector.tensor_scalar(msk2[:], L2[:], mx2[:], None, AL.is_equal)
                mi2 = spool.tile([128, E], f32, tag="mi2")
                nc.vector.tensor_tensor(mi2[:], msk2[:], i8m_sb[:], AL.mult)
                nc.vector.tensor_reduce(svals[:, 1:2], mi2[:],
                                        mybir.AxisListType.X, AL.max)
                nc.scalar.activation(evals[:, 1:2], mx2[:], AF.Exp)
                evs[tt], svs[tt] = evals, svals

            for tt in range(2):
                evals, svals = evs[tt], svs[tt]
                # transpose evals/svals -> rows
                psT = pspool.tile([2, 128], f32, tag="ps_small", bufs=1,
                                  name="psT")
                nc.tensor.transpose(psT[:], evals[:], ident[:])
                nc.vector.tensor_copy(ev_rows[:, tt * 128:(tt + 1) * 128], psT[:])
                psT2 = pspool.tile([2, 128], f32, tag="ps_small", bufs=1,
                                   name="psT2")
                nc.tensor.transpose(psT2[:], svals[:], ident[:])
                nc.vector.tensor_copy(s_rows[:, tt * 128:(tt + 1) * 128], psT2[:])

            # partition-replicated AllReduce payload [128, 4] with columns
            # (batch, slot): ar[p, 2b+s] = sum_t exp_s(t). The post-AR
            # consumer is then pure DVE (no PE op ever waits on the AR).
            denc = cpool.tile([2, 1], f32, name="denc")
            nc.vector.tensor_reduce(denc[:], ev_rows[:], mybir.AxisListType.X,
                                    AL.add)
            bden = cpool.tile([2, 4], f32, name="bden")
            nc.vector.tensor_scalar(bden[:], bscat_sb[:], denc[:], None,
                                    AL.mult)
            psA2 = pspool.tile([128, 4], f32, tag="ps_small", bufs=1,
                               name="psA2")
            nc.tensor.matmul(psA2[:], ones2[:], bden[:], start=True, stop=True)
            ar_sb = cpool.tile([128, 4], f32, name="ar_sb")
            nc.vector.tensor_copy(ar_sb[:], psA2[:])
            ar_in = drpool.tile([128, 4], f32, name="ar_in")
            ar_out = drpool.tile([128, 4], f32, name="ar_out",
                                 addr_space="Shared")
            nc.gpsimd.dma_start(out=ar_in[:], in_=ar_sb[:])
            if SKIP_AR:
                nc.gpsimd.dma_start(out=ar_out[:], in_=ar_in[:])
            else:
                nc.gpsimd.collective_compute(
                    "AllReduce", AL.add,
                    replica_groups=[list(range(NCORES))],
                    ins=[ar_in.opt()], outs=[ar_out.opt()])
            den2b = cpool.tile([128, 4], f32, name="den2b")
            nc.gpsimd.dma_start(out=den2b[:], in_=ar_out[:])

            # ---- AR-independent prep: masks, unnormalized weight rows ----
            # broadcast slot rows along partitions via K=2 matmul with a
            # row-selector constant (sel2[:, j*128:(j+1)*128] has row j = 1)
            for j in range(2):
                psM = pspool.tile([128, TC], f32, tag="psV", name="psM")
                nc.tensor.matmul(psM[:], sel2_sb[:, j * 128:(j + 1) * 128],
                                 s_rows[:], start=True, stop=True)
                nc.vector.tensor_scalar(Mj[:, j * TC:(j + 1) * TC], psM[:],
                                        eid_sb[:], None, AL.is_equal)
                psB = pspool.tile([128, TC], f32, tag="psV", name="psB")
                nc.tensor.matmul(psB[:], sel2_sb[:, j * 128:(j + 1) * 128],
                                 ev_rows[:], start=True, stop=True)
                nc.vector.tensor_copy(cbraw[:, j * TC:(j + 1) * TC], psB[:])
            for j in range(2):
                nc.vector.tensor_tensor(Ut[:, j * TC:(j + 1) * TC], UA[:],
                                        Mj[:, j * TC:(j + 1) * TC], AL.mult)
                nc.vector.tensor_tensor(Gt[:, j * TC:(j + 1) * TC], GA[:],
                                        Mj[:, j * TC:(j + 1) * TC], AL.mult)

            # ---- phases 2+5+6: h-tile loop ----
            psV = pspool.tile([128, 2 * TC], f32, tag="psV", name="psV")
            HH = HT // 2 * 128          # half of the wd h columns (2816)
            wd_pre = {}                 # (di, half) -> prefetched tile
            pend_v = []                 # delayed psV matmuls [(dA_t, hh_pair)]
            PEND = 1                    # psV deferred this many h-tiles
            CBL = 8                     # cb application deferred this many
            ch_defer = []               # (hh_pair, i)
            psv_started = [False]

            def flush_v(last=False):
                while pend_v and (last or len(pend_v) > PEND):
                    pv_dA, pv_ch = pend_v.pop(0)
                    stop = last and not pend_v
                    nc.tensor.matmul(psV[:], pv_dA[:], pv_ch[:],
                                     start=not psv_started[0], stop=stop,
                                     skip_group_check=True)
                    psv_started[0] = True

            def load_wd(di, h):
                # wd streams on the Activation queue: the SP queue is issue-
                # rate-bound (~0.9us per dma_start) feeding wu/wg
                # bufs=3 is the prefetch: three halves fill early and the
                # 4th DMA self-throttles until the down phase starts reading
                t = wpool.tile([128, HH], bf16, tag="wd", bufs=3, name="wd_t")
                nc.scalar.dma_start(
                    out=t[:], in_=d_wd[di][:, h * HH:(h + 1) * HH])
                return t

            for i in range(HT):
                wu_t = wpool.tile([128, KT * 128], bf16, tag="wu", bufs=3)
                nc.sync.dma_start(out=wu_t[:], in_=d_wu[i])
                wg_t = wpool.tile([128, KT * 128], bf16, tag="wg", bufs=3)
                nc.sync.dma_start(out=wg_t[:], in_=d_wg[i])
                Bp_t = bpool.tile([128, 384], bf16, tag="Bp", bufs=10)
                nc.scalar.dma_start(out=Bp_t[:], in_=d_Bp[i])
                uB_t = Bp_t[:, 0:128]
                gB_t = Bp_t[:, 128:256]
                dA_t = Bp_t[:, 256:384]

                psUG = pspool.tile([128, 2 * TC], f32, tag="psUG", bufs=2,
                                   name="psUG")
                for k in range(KT):
                    nc.tensor.matmul(psUG[:, 0:TC],
                                     wu_t[:, k * 128:(k + 1) * 128],
                                     xTb[:, k * TC:(k + 1) * TC],
                                     start=(k == 0), stop=(k == KT - 1))
                for k in range(KT):
                    nc.tensor.matmul(psUG[:, TC:2 * TC],
                                     wg_t[:, k * 128:(k + 1) * 128],
                                     xTb[:, k * TC:(k + 1) * TC],
                                     start=(k == 0), stop=(k == KT - 1))
                flush_v()
                U_sb = spool.tile([128, TC], f32, tag="U_sb")
                nc.scalar.copy(U_sb[:], psUG[:, 0:TC])
                G_sb = spool.tile([128, TC], f32, tag="G_sb")
                nc.scalar.copy(G_sb[:], psUG[:, TC:2 * TC])

                psLO = pspool.tile([128, 4 * TC], f32, tag="psLO", bufs=2,
                                   name="psLO")
                # all four B-proj matmuls are emitted BEFORE any DVE consumer
                # of psLO: the tile-granular WAR tracking otherwise stalls the
                # in-order PE queue on slot-0's DVE reads (663ns/iter)
                for j in range(2):
                    nc.tensor.matmul(psLO[:, (2 * j) * TC:(2 * j + 1) * TC],
                                     uB_t[:],
                                     Ut[:, j * TC:(j + 1) * TC],
                                     start=True, stop=True)
                    nc.tensor.matmul(psLO[:, (2 * j + 1) * TC:(2 * j + 2) * TC],
                                     gB_t[:],
                                     Gt[:, j * TC:(j + 1) * TC],
                                     start=True, stop=True)
                # both slots' c*h in ONE tile so the down_A contraction is a
                # single [128,512] matmul per h-tile
                # unweighted hh for both slots in ONE tile: the down_A
                # contraction commutes with the per-token routing weight
                # (psV_raw·cb == psV of weighted ch), so neither psV nor any
                # other PE op ever depends on the AllReduce
                hh_pair = spool.tile([128, 2 * TC], bf16, tag="hhp", bufs=12)
                tus, tgs = [], []
                for j in range(2):
                    tu = spool.tile([128, TC], bf16, tag="tu")
                    nc.vector.tensor_tensor(
                        tu[:], U_sb[:], psLO[:, (2 * j) * TC:(2 * j + 1) * TC],
                        AL.add)
                    tg = spool.tile([128, TC], bf16, tag="tg")
                    nc.vector.tensor_tensor(
                        tg[:], G_sb[:],
                        psLO[:, (2 * j + 1) * TC:(2 * j + 2) * TC], AL.add)
                    tus.append(tu)
                    tgs.append(tg)
                for j in range(2):
                    su = spool.tile([128, TC], bf16, tag="su")
                    nc.scalar.activation(su[:], tus[j][:], AF.Silu)
                    nc.vector.tensor_tensor(hh_pair[:, j * TC:(j + 1) * TC],
                                            su[:], tgs[j][:], AL.mult)
                pend_v.append((dA_t, hh_pair))
                ch_defer.append((hh_pair, i))
                if i == CBL:
                    # post-AR path, all DVE, emitted after CBL iterations of
                    # eager work: only the deferred mixed flushes below (and
                    # the final vt scaling) consume the AllReduce result
                    mden = cpool.tile([128, 4], f32, name="mden")
                    nc.vector.tensor_tensor(mden[:], den2b[:], bmask_sb[:],
                                            AL.mult)
                    myden = cpool.tile([128, 2], f32, name="myden")
                    nc.vector.tensor_tensor(myden[:], mden[:, 0:2],
                                            mden[:, 2:4], AL.add)
                    rcp2 = cpool.tile([128, 2], f32, name="rcp2")
                    nc.vector.reciprocal(rcp2[:], myden[:])
                    for j in range(2):
                        nc.vector.tensor_scalar(cb[:, j * TC:(j + 1) * TC],
                                                cbraw[:, j * TC:(j + 1) * TC],
                                                rcp2[:, j:j + 1], None,
                                                AL.mult)
                if i >= CBL:
                    flush_mixed()

            while ch_defer:
                flush_mixed()
            flush_v(last=True)
            # masked v
            for j in range(2):
                nc.vector.tensor_tensor(vt[:, j * TC:(j + 1) * TC],
                                        psV[:, j * TC:(j + 1) * TC],
                                        Mj[:, j * TC:(j + 1) * TC], AL.mult)

            if DEBUG_TAPS:
                for nm, tl in [("crows", crows), ("srows", s_rows),
                               ("cb", cb), ("Mj", Mj), ("UA", UA),
                               ("GA", GA), ("vt", vt),
                               ("mixed0", mixed[:, 0:TC]),
                               ("mixed7", mixed[:, 7 * TC:8 * TC])]:
                    shp = [tl.shape[0], tl.shape[-1]]
                    dbg = nc.dram_tensor(f"dbg_{nm}", shp, f32,
                                         kind="ExternalOutput").ap()
                    nc.sync.dma_start(out=dbg[:], in_=tl[:])

            # ---- phase 7: down GEMM + LoRA-down ----
            for di in range(DT):
                wd_h = [wd_pre.get((di, h)) or load_wd(di, h)
                        for h in range(2)]
                psO = pspool.tile([128, TC], f32, tag="psUG", bufs=2, name="psO")
                for hk in range(HT):
                    w = wd_h[hk // 22][:, (hk % 22) * 128:(hk % 22 + 1) * 128]
                    nc.tensor.matmul(psO[:], w,
                                     mixed[:, hk * TC:(hk + 1) * TC],
                                     start=(hk == 0), stop=False,
                                     skip_group_check=True)
                nc.tensor.matmul(psO[:], dB_sb[:, di * 128:(di + 1) * 128],
                                 vt[:, 0:TC], start=False, stop=False,
                                 skip_group_check=True)
                nc.tensor.matmul(psO[:], dB_sb[:, di * 128:(di + 1) * 128],
                                 vt[:, TC:2 * TC], start=False, stop=True,
                                 skip_group_check=True)
                o_sb = spool.tile([128, TC], bf16, tag="o_sb")
                nc.scalar.copy(o_sb[:], psO[:])
                nc.sync.dma_start(out=d_out[di * 128:(di + 1) * 128, :],
                                  in_=o_sb[:])

    nc.compile()
    return nc


def _prep_shared(inputs):
    """Host-side layout prep of weight tensors (shared across cores)."""
    import ml_dtypes
    bf16 = np.dtype(ml_dtypes.bfloat16)
    f32 = np.float32

    def c(a, dt):
        return np.ascontiguousarray(a.astype(dt, copy=False))

    w_up, w_gate, w_down = inputs["w_up"], inputs["w_gate"], inputs["w_down"]
    wu = c(w_up.reshape(HT, 128, KT, 128).transpose(0, 3, 2, 1)
           .reshape(HT, 128, KT * 128), bf16)
    wg = c(w_gate.reshape(HT, 128, KT, 128).transpose(0, 3, 2, 1)
           .reshape(HT, 128, KT * 128), bf16)
    wd = c(w_down.reshape(DT, 128, HT, 128).transpose(0, 3, 2, 1)
           .reshape(DT, 128, HT * 128), bf16)

    A_stack = np.concatenate([
        inputs["up_A"].reshape(ER, D),
        inputs["gate_A"].reshape(ER, D)], axis=0)          # [2*ER, D]
    # Ah[p, k*2ER + m] = A_stack[m, k*128+p]
    Ah = c(A_stack.reshape(2 * ER, KT, 128).transpose(2, 1, 0)
           .reshape(128, KT * 2 * ER), bf16)

    up_B_all = (inputs["up_B"].transpose(0, 2, 1).reshape(ER, H)
                * ALPHA).astype(f32)
    gate_B_all = (inputs["gate_B"].transpose(0, 2, 1).reshape(ER, H)
                  * ALPHA).astype(f32)
    uB = up_B_all.reshape(ER, HT, 128).transpose(1, 0, 2)
    gB = gate_B_all.reshape(ER, HT, 128).transpose(1, 0, 2)

    down_A_all = inputs["down_A"].reshape(ER, H).astype(f32)
    dA = down_A_all.T.reshape(HT, 128, ER)
    # one [HT, 128, 384] pack: up_B | gate_B | down_A per h-tile
    Bp = c(np.concatenate([uB, gB, dA], axis=2), bf16)
    down_B_all = (inputs["down_B"].transpose(0, 2, 1).reshape(ER, D)
                  * ALPHA).astype(f32)
    dB = c(down_B_all, bf16)

    gate_wT = inputs["gate_w"].T.astype(f32)               # [D, E]
    gw = c(gate_wT.reshape(KT, 128, E).transpose(1, 0, 2)
           .reshape(128, KT * E), f32)

    eid = (8.0 - (np.arange(128) // R)).astype(f32).reshape(128, 1)
    i8m = np.tile((8.0 - np.arange(E)).astype(f32), (128, 1))
    ei = c(np.concatenate([eid, i8m], axis=1), f32)
    sel2 = np.zeros((2, 256), f32)
    sel2[0, 0:128] = 1.0
    sel2[1, 128:256] = 1.0

    return dict(wu=wu, wg=wg, wd=wd, Ah=Ah, Bp=Bp, dB=dB,
                gw=gw, ei=ei, sel2=sel2)


def kernel(**inputs):
    import ml_dtypes
    from concourse.bass_utils import run_bass_kernel_spmd

    bf16 = np.dtype(ml_dtypes.bfloat16)
    inputs = {k: np.asarray(v) for k, v in inputs.items()}
    if "nc" not in _cache:
        _cache["nc"] = _build()
    nc = _cache["nc"]

    shared = _prep_shared(inputs)
    x = inputs["x"].astype(np.float32)
    xt = x.reshape(T, D)

    in_maps = []
    for cix in range(NCORES):
        xc = xt[cix * TC:(cix + 1) * TC]                   # [TC, D]
        xT = xc.T                                          # [D, TC] f32
        # sb layout [128, KT*TC]: sb[p, k*TC+t] = x[k*128+p, t]
        xTs = np.ascontiguousarray(
            xT.reshape(KT, 128, TC).transpose(1, 0, 2).reshape(128, KT * TC))
        b = (cix * TC) // S
        bscat = np.zeros((2, 4), np.float32)
        bscat[0, 2 * b] = 1.0
        bscat[1, 2 * b + 1] = 1.0
        bmask = np.zeros((128, 4), np.float32)
        bmask[:, 2 * b:2 * b + 2] = 1.0
        m = dict(shared)
        m["xT"] = xTs
        m["xTb"] = np.ascontiguousarray(xTs.astype(bf16))
        m["bscat"] = bscat
        m["bmask"] = bmask
        in_maps.append(m)

    res = run_bass_kernel_spmd(nc, in_maps, list(range(NCORES)))
    out = np.empty((T, D), np.float32)
    for cix in range(NCORES):
        out[cix * TC:(cix + 1) * TC, :] = res.results[cix]["outT"].T
    return out.reshape(B, S, D)


# revision 57
# speedup vs baseline: 1.3880x; 1.0359x over previous
"""Trainium2 Bass kernel for nn_MistralMoLoraLayer (MoE-routed LoRA FFN).

Strategy: data-parallel over tokens (8 cores x 256 tokens), base FFN weights
replicated, all-expert LoRA replicated. The per-(batch,slot) softmax over the
sequence axis needs global denominators -> tiny [2,8] AllReduce.

Per-core math (all tiles [h/er/d partitions, tokens free]):
  router: logits = x @ gate_w.T (f32); top-2 (value,index) per token; exp; AR
          of per-batch-slot sums; weights w_j = exp_j / denom[batch, j]
  A-proj: UA/GA [E*R=128, t] = stacked up_A/gate_A @ x.T   (one K=128 chain)
  slot-mask trick: Ut_j = UA * M_j where M_j[e*R+r, t] = (sel_j(t)==e);
          lo_up_j[h,t] = (stacked up_B) @ Ut_j  == up_B[sel_j(t)] @ u_{sel_j(t)}
  h_j = silu(U + lo_up_j) * (G + lo_gate_j); ch_j = c_j * h_j
  mixed = ch_0 + ch_1
  v_j[er,t] = (stacked down_A) @ ch_j  (accumulated over h), masked by M_j
  outT[d,t] = w_down-chain @ mixed + (stacked down_B) @ v_0 + ... @ v_1

Everything off the router path runs in bf16 (weights streamed bf16, PSUM
accumulation f32); the router logits stay f32 so near-tie top-2 selections
match the reference.
"""

import numpy as np

# problem constants (hardcoded; kernel.py must be self-contained)
B, S, D, H, E, R, TOPK = 2, 1024, 2048, 5632, 8, 16, 2
ALPHA = 2.0
T = B * S
NCORES = 8
TC = T // NCORES           # 256 tokens per core
KT = D // 128              # 16 k-tiles over D
HT = H // 128              # 44 h-tiles
DT = D // 128              # 16 d-tiles
ER = E * R                 # 128

DEBUG_TAPS = False         # add intermediate-tensor outputs for debugging
SKIP_AR = False            # replace AllReduce with local copy (for TimelineSim)
WD_PRE = 6                 # wd halves prefetched during the h-loop

_cache = {}


def _build():
    import concourse.bacc as bacc
    import concourse.bass as bass
    import concourse.mybir as mybir
    import concourse.tile as tile
    from concourse.masks import make_identity

    f32 = mybir.dt.float32
    bf16 = mybir.dt.bfloat16
    AL = mybir.AluOpType
    AF = mybir.ActivationFunctionType

    nc = bacc.Bacc("TRN2", target_bir_lowering=False, debug=False,
                   num_devices=NCORES)

    # ---- DRAM I/O (host-prepped layouts; single-DMA where possible) ----
    # x ships as bf16 hi + bf16 lo (x = hi + lo to ~fp24): the router runs
    # three bf16 chains (hi*gwh + lo*gwh + hi*gwl), giving logits accurate to
    # ~5e-6 -- far inside the 1.3e-4 minimum top-2 margin of this input
    d_xTb = nc.dram_tensor("xTb", [128, KT * TC], bf16, kind="ExternalInput").ap()
    d_xlo = nc.dram_tensor("xlo", [128, KT * TC], bf16, kind="ExternalInput").ap()
    # cols 0:KT*E = gate_w hi, KT*E:2*KT*E = gate_w lo
    d_gwp = nc.dram_tensor("gwp", [128, 2 * KT * E], bf16, kind="ExternalInput").ap()
    d_wu = nc.dram_tensor("wu", [HT, 128, KT * 128], bf16, kind="ExternalInput").ap()
    d_wg = nc.dram_tensor("wg", [HT, 128, KT * 128], bf16, kind="ExternalInput").ap()
    d_wd = nc.dram_tensor("wd", [DT, 128, HT * 128], bf16, kind="ExternalInput").ap()
    d_A = nc.dram_tensor("Ah", [128, KT * 2 * ER], bf16, kind="ExternalInput").ap()
    # per-h-tile LoRA pack: cols 0:128 up_B, 128:256 gate_B, 256:384 down_A
    d_Bp = nc.dram_tensor("Bp", [HT, 128, 384], bf16, kind="ExternalInput").ap()
    d_dB = nc.dram_tensor("dB", [128, D], bf16, kind="ExternalInput").ap()
    # col 0 = eid, cols 1:9 = i8m
    d_ei = nc.dram_tensor("ei", [128, 1 + E], f32, kind="ExternalInput").ap()
    # bscat[s, 2b+s] = 1 for this core's batch b; bmask = 1 at cols 2b,2b+1
    d_bscat = nc.dram_tensor("bscat", [2, 4], f32, kind="ExternalInput").ap()
    d_bmask = nc.dram_tensor("bmask", [128, 4], f32, kind="ExternalInput").ap()
    d_sel2 = nc.dram_tensor("sel2", [2, 256], f32, kind="ExternalInput").ap()
    d_out = nc.dram_tensor("outT", [D, TC], bf16, kind="ExternalOutput").ap()

    with tile.TileContext(nc) as tc:
        import contextlib
        ctx = contextlib.ExitStack()
        with ctx:
            cpool = ctx.enter_context(tc.tile_pool(name="const", bufs=1))
            wpool = ctx.enter_context(tc.tile_pool(name="wstream", bufs=2))
            bpool = ctx.enter_context(tc.tile_pool(name="bstream", bufs=3))
            spool = ctx.enter_context(tc.tile_pool(name="work", bufs=2))
            pspool = ctx.enter_context(
                tc.tile_pool(name="ps", bufs=1, space="PSUM"))
            drpool = ctx.enter_context(
                tc.tile_pool(name="dram", bufs=1, space="DRAM"))

            # ---- DMA plan: SP queue carries x + the wu/wg stream; the
            # Activation queue carries everything else (the SP sequencer
            # costs ~0.9us per dma_start, so queue assignment matters) ----
            xTb = cpool.tile([128, KT * TC], bf16, name="xTb")
            nc.sync.dma_start(out=xTb[:], in_=d_xTb[:])
            xlo = cpool.tile([128, KT * TC], bf16, name="xlo")
            nc.sync.dma_start(out=xlo[:], in_=d_xlo[:])

            # warm both activation tables first thing on the Act engine (Silu
            # then Exp, so Exp is resident for the router; Silu reloads once
            # at h-loop start, off the AR critical path)
            warm = cpool.tile([1, 2], f32, name="warm")
            nc.vector.memset(warm, 0.0)
            nc.scalar.activation(warm[:, 0:1], warm[:, 0:1], AF.Silu)
            nc.scalar.activation(warm[:, 1:2], warm[:, 1:2], AF.Exp)

            gwp_sb = cpool.tile([128, 2 * KT * E], bf16, name="gwp_sb")
            nc.scalar.dma_start(out=gwp_sb[:], in_=d_gwp[:])
            ei_sb = cpool.tile([128, 1 + E], f32, name="ei_sb")
            nc.scalar.dma_start(out=ei_sb[:], in_=d_ei[:])
            eid_sb = ei_sb[:, 0:1]
            i8m_sb = ei_sb[:, 1:1 + E]
            bscat_sb = cpool.tile([2, 4], f32, name="bscat_sb")
            nc.scalar.dma_start(out=bscat_sb[:], in_=d_bscat[:])
            bmask_sb = cpool.tile([128, 4], f32, name="bmask_sb")
            nc.scalar.dma_start(out=bmask_sb[:], in_=d_bmask[:])
            sel2_sb = cpool.tile([2, 256], f32, name="sel2_sb")
            nc.scalar.dma_start(out=sel2_sb[:], in_=d_sel2[:])
            A_sb = cpool.tile([128, KT * 2 * ER], bf16, name="A_sb")
            nc.scalar.dma_start(out=A_sb[:], in_=d_A[:])
            dB_sb = cpool.tile([128, D], bf16, name="dB_sb")
            nc.scalar.dma_start(out=dB_sb[:], in_=d_dB[:])

            ident = cpool.tile([128, 128], f32, name="ident")
            make_identity(nc, ident)
            ones_col = cpool.tile([128, 1], f32, name="ones_col")
            nc.vector.memset(ones_col, 1.0)
            ones_row = cpool.tile([1, 128], f32, name="ones_row")
            nc.vector.memset(ones_row, 1.0)
            ones2 = cpool.tile([2, 128], f32, name="ones2")
            nc.vector.memset(ones2, 1.0)
            ones128_2 = cpool.tile([128, 2], f32, name="ones128_2")
            nc.vector.memset(ones128_2, 1.0)

            mixed = cpool.tile([128, HT * TC], bf16, name="mixed")
            ev_rows = cpool.tile([2, TC], f32, name="ev_rows")
            s_rows = cpool.tile([2, TC], f32, name="s_rows")
            cbraw = cpool.tile([128, 2 * TC], f32, name="cbraw")
            cb = cpool.tile([128, 2 * TC], bf16, name="cb")
            Mj = cpool.tile([128, 2 * TC], f32, name="Mj")
            UA = cpool.tile([128, TC], f32, name="UA")
            GA = cpool.tile([128, TC], f32, name="GA")
            Ut = cpool.tile([128, 2 * TC], bf16, name="Ut")
            Gt = cpool.tile([128, 2 * TC], bf16, name="Gt")
            vt = cpool.tile([128, 2 * TC], bf16, name="vt")

            # ---- phase 0: stacked A-projections (xTb lands before the f32
            # router x, so these PE chains fill the front-end DMA wait) ----
            psUA = pspool.tile([128, TC], f32, tag="psV", name="psUA")
            for k in range(KT):
                nc.tensor.matmul(psUA[:],
                                 A_sb[:, k * 2 * ER: k * 2 * ER + ER],
                                 xTb[:, k * TC:(k + 1) * TC],
                                 start=(k == 0), stop=(k == KT - 1))
            nc.vector.tensor_copy(UA[:], psUA[:])
            psGA = pspool.tile([128, TC], f32, tag="psV", name="psGA")
            for k in range(KT):
                nc.tensor.matmul(psGA[:],
                                 A_sb[:, k * 2 * ER + ER:(k + 1) * 2 * ER],
                                 xTb[:, k * TC:(k + 1) * TC],
                                 start=(k == 0), stop=(k == KT - 1))
            nc.vector.tensor_copy(GA[:], psGA[:])

            # ---- phase 1a: router logit chains (PE, fp24 via hi/lo) ----
            GWH = KT * E
            psL = {}
            for tt in range(2):
                psL[tt] = pspool.tile([128, TC], f32, tag="psUG", bufs=2,
                                      name=f"psL{tt}")
                passes = [(xTb, 0), (xlo, 0), (xTb, GWH)]
                for pi, (xa, go) in enumerate(passes):
                    for k in range(KT):
                        nc.tensor.matmul(
                            psL[tt][:, 0:E],
                            xa[:, k * TC + tt * 128: k * TC + tt * 128 + 128],
                            gwp_sb[:, go + k * E:go + (k + 1) * E],
                            start=(pi == 0 and k == 0),
                            stop=(pi == 2 and k == KT - 1))

            # ---- phase 1b: top-2 select + exp (both token-tiles overlap) ----
            den_parts = cpool.tile([1, 4], f32, name="den_parts")
            evs, svs = {}, {}
            for tt in range(2):
                L = spool.tile([128, E], f32, tag="L")
                nc.vector.tensor_copy(L[:], psL[tt][:, 0:E])
                mx1 = spool.tile([128, 1], f32, tag="mx1")
                nc.vector.tensor_reduce(mx1[:], L[:], mybir.AxisListType.X, AL.max)
                msk = spool.tile([128, E], f32, tag="msk")
                nc.vector.tensor_scalar(msk[:], L[:], mx1[:], None, AL.is_equal)
                mi = spool.tile([128, E], f32, tag="mi")
                nc.vector.tensor_tensor(mi[:], msk[:], i8m_sb[:], AL.mult)
                svals = spool.tile([128, 2], f32, tag="svals")
                nc.vector.tensor_reduce(svals[:, 0:1], mi[:],
                                        mybir.AxisListType.X, AL.max)
                evals = spool.tile([128, 2], f32, tag="evals")
                nc.scalar.activation(evals[:, 0:1], mx1[:], AF.Exp)
                # mask out slot-0 winner, find second max
                big = spool.tile([128, E], f32, tag="big")
                nc.vector.tensor_scalar(big[:], msk[:], 1e30, None, AL.mult)
                L2 = spool.tile([128, E], f32, tag="L2")
                nc.vector.tensor_tensor(L2[:], L[:], big[:], AL.subtract)
                mx2 = spool.tile([128, 1], f32, tag="mx2")
                nc.vector.tensor_reduce(mx2[:], L2[:], mybir.AxisListType.X, AL.max)
                msk2 = spool.tile([128, E], f32, tag="msk2")
                nc.vector.tensor_scalar(msk2[:], L2[:], mx2[:], None, AL.is_equal)
                mi2 = spool.tile([128, E], f32, tag="mi2")
                nc.vector.tensor_tensor(mi2[:], msk2[:], i8m_sb[:], AL.mult)
                nc.vector.tensor_reduce(svals[:, 1:2], mi2[:],
                                        mybir.AxisListType.X, AL.max)
                nc.scalar.activation(evals[:, 1:2], mx2[:], AF.Exp)
                evs[tt], svs[tt] = evals, svals

            for tt in range(2):
                evals, svals = evs[tt], svs[tt]
                # transpose evals/svals -> rows
                psT = pspool.tile([2, 128], f32, tag="ps_small", bufs=1,
                                  name="psT")
                nc.tensor.transpose(psT[:], evals[:], ident[:])
                nc.vector.tensor_copy(ev_rows[:, tt * 128:(tt + 1) * 128], psT[:])
                psT2 = pspool.tile([2, 128], f32, tag="ps_small", bufs=1,
                                   name="psT2")
                nc.tensor.transpose(psT2[:], svals[:], ident[:])
                nc.vector.tensor_copy(s_rows[:, tt * 128:(tt + 1) * 128], psT2[:])

            # partition-replicated AllReduce payload [128, 4] with columns
            # (batch, slot): ar[p, 2b+s] = sum_t exp_s(t). The post-AR
            # consumer is then pure DVE (no PE op ever waits on the AR).
            denc = cpool.tile([2, 1], f32, name="denc")
            nc.vector.tensor_reduce(denc[:], ev_rows[:], mybir.AxisListType.X,
                                    AL.add)
            bden = cpool.tile([2, 4], f32, name="bden")
            nc.vector.tensor_scalar(bden[:], bscat_sb[:], denc[:], None,
                                    AL.mult)
            psA2 = pspool.tile([128, 4], f32, tag="ps_small", bufs=1,
                               name="psA2")
            nc.tensor.matmul(psA2[:], ones2[:], bden[:], start=True, stop=True)
            ar_sb = cpool.tile([128, 4], f32, name="ar_sb")
            nc.vector.tensor_copy(ar_sb[:], psA2[:])
            ar_in = drpool.tile([128, 4], f32, name="ar_in")
            ar_out = drpool.tile([128, 4], f32, name="ar_out",
                                 addr_space="Shared")
            nc.gpsimd.dma_start(out=ar_in[:], in_=ar_sb[:])
            if SKIP_AR:
                nc.gpsimd.dma_start(out=ar_out[:], in_=ar_in[:])
            else:
                nc.gpsimd.collective_compute(
                    "AllReduce", AL.add,
                    replica_groups=[list(range(NCORES))],
                    ins=[ar_in.opt()], outs=[ar_out.opt()])
            den2b = cpool.tile([128, 4], f32, name="den2b")
            nc.gpsimd.dma_start(out=den2b[:], in_=ar_out[:])

            # ---- AR-independent prep: masks, unnormalized weight rows ----
            # broadcast slot rows along partitions via K=2 matmul with a
            # row-selector constant (sel2[:, j*128:(j+1)*128] has row j = 1)
            for j in range(2):
                psM = pspool.tile([128, TC], f32, tag="psV", name="psM")
                nc.tensor.matmul(psM[:], sel2_sb[:, j * 128:(j + 1) * 128],
                                 s_rows[:], start=True, stop=True)
                nc.vector.tensor_scalar(Mj[:, j * TC:(j + 1) * TC], psM[:],
                                        eid_sb[:], None, AL.is_equal)
                psB = pspool.tile([128, TC], f32, tag="psV", name="psB")
                nc.tensor.matmul(psB[:], sel2_sb[:, j * 128:(j + 1) * 128],
                                 ev_rows[:], start=True, stop=True)
                nc.vector.tensor_copy(cbraw[:, j * TC:(j + 1) * TC], psB[:])
            for j in range(2):
                nc.vector.tensor_tensor(Ut[:, j * TC:(j + 1) * TC], UA[:],
                                        Mj[:, j * TC:(j + 1) * TC], AL.mult)
                nc.vector.tensor_tensor(Gt[:, j * TC:(j + 1) * TC], GA[:],
                                        Mj[:, j * TC:(j + 1) * TC], AL.mult)

            # ---- phases 2+5+6: h-tile loop ----
            psV = pspool.tile([128, 2 * TC], f32, tag="psV", name="psV")
            HH = HT // 2 * 128          # half of the wd h columns (2816)
            wd_pre = {}                 # (di, half) -> prefetched tile
            pend_v = []                 # delayed psV matmuls [(dA_t, hh_pair)]
            PEND = 1                    # psV deferred this many h-tiles
            CBL = 8                     # cb application deferred this many
            ch_defer = []               # (hh_pair, i)
            psv_started = [False]

            def flush_v(last=False):
                while pend_v and (last or len(pend_v) > PEND):
                    pv_dA, pv_hh = pend_v.pop(0)
                    stop = last and not pend_v
                    nc.tensor.matmul(psV[:], pv_dA[:], pv_hh[:],
                                     start=not psv_started[0], stop=stop,
                                     skip_group_check=True)
                    psv_started[0] = True

            def flush_mixed():
                # on GpSimd: every AR-dependent elementwise op lives on the
                # otherwise-idle Pool queue, so the eager DVE/Act/PE streams
                # never block on the AllReduce no matter how the scheduler
                # orders them
                f_hh, fi = ch_defer.pop(0)
                c0 = spool.tile([128, TC], bf16, tag="ct")
                nc.gpsimd.tensor_tensor(c0[:], f_hh[:, 0:TC], cb[:, 0:TC],
                                        AL.mult)
                c1 = spool.tile([128, TC], bf16, tag="ct")
                nc.gpsimd.tensor_tensor(c1[:], f_hh[:, TC:2 * TC],
                                        cb[:, TC:2 * TC], AL.mult)
                nc.gpsimd.tensor_tensor(mixed[:, fi * TC:(fi + 1) * TC],
                                        c0[:], c1[:], AL.add)

            def load_wd(di, h):
                # wd streams on the Activation queue: the SP queue is issue-
                # rate-bound (~0.9us per dma_start) feeding wu/wg
                # bufs=3 is the prefetch: three halves fill early and the
                # 4th DMA self-throttles until the down phase starts reading
                t = wpool.tile([128, HH], bf16, tag="wd", bufs=3, name="wd_t")
                nc.scalar.dma_start(
                    out=t[:], in_=d_wd[di][:, h * HH:(h + 1) * HH])
                return t

            for i in range(HT):
                wu_t = wpool.tile([128, KT * 128], bf16, tag="wu", bufs=3)
                nc.sync.dma_start(out=wu_t[:], in_=d_wu[i])
                wg_t = wpool.tile([128, KT * 128], bf16, tag="wg", bufs=3)
                nc.sync.dma_start(out=wg_t[:], in_=d_wg[i])
                Bp_t = bpool.tile([128, 384], bf16, tag="Bp", bufs=10)
                nc.scalar.dma_start(out=Bp_t[:], in_=d_Bp[i])
                uB_t = Bp_t[:, 0:128]
                gB_t = Bp_t[:, 128:256]
                dA_t = Bp_t[:, 256:384]

                psUG = pspool.tile([128, 2 * TC], f32, tag="psUG", bufs=2,
                                   name="psUG")
                for k in range(KT):
                    nc.tensor.matmul(psUG[:, 0:TC],
                                     wu_t[:, k * 128:(k + 1) * 128],
                                     xTb[:, k * TC:(k + 1) * TC],
                                     start=(k == 0), stop=(k == KT - 1))
                for k in range(KT):
                    nc.tensor.matmul(psUG[:, TC:2 * TC],
                                     wg_t[:, k * 128:(k + 1) * 128],
                                     xTb[:, k * TC:(k + 1) * TC],
                                     start=(k == 0), stop=(k == KT - 1))
                flush_v()
                U_sb = spool.tile([128, TC], f32, tag="U_sb")
                nc.scalar.copy(U_sb[:], psUG[:, 0:TC])
                G_sb = spool.tile([128, TC], f32, tag="G_sb")
                nc.scalar.copy(G_sb[:], psUG[:, TC:2 * TC])

                psLO = pspool.tile([128, 4 * TC], f32, tag="psLO", bufs=2,
                                   name="psLO")
                # all four B-proj matmuls are emitted BEFORE any DVE consumer
                # of psLO: the tile-granular WAR tracking otherwise stalls the
                # in-order PE queue on slot-0's DVE reads (663ns/iter)
                for j in range(2):
                    nc.tensor.matmul(psLO[:, (2 * j) * TC:(2 * j + 1) * TC],
                                     uB_t[:],
                                     Ut[:, j * TC:(j + 1) * TC],
                                     start=True, stop=True)
                    nc.tensor.matmul(psLO[:, (2 * j + 1) * TC:(2 * j + 2) * TC],
                                     gB_t[:],
                                     Gt[:, j * TC:(j + 1) * TC],
                                     start=True, stop=True)
                # both slots' c*h in ONE tile so the down_A contraction is a
                # single [128,512] matmul per h-tile
                # unweighted hh for both slots in ONE tile: the down_A
                # contraction commutes with the per-token routing weight
                # (psV_raw·cb == psV of weighted ch), so neither psV nor any
                # other PE op ever depends on the AllReduce
                hh_pair = spool.tile([128, 2 * TC], bf16, tag="hhp", bufs=12)
                tus, tgs = [], []
                for j in range(2):
                    tu = spool.tile([128, TC], bf16, tag="tu")
                    nc.vector.tensor_tensor(
                        tu[:], U_sb[:], psLO[:, (2 * j) * TC:(2 * j + 1) * TC],
                        AL.add)
                    tg = spool.tile([128, TC], bf16, tag="tg")
                    nc.vector.tensor_tensor(
                        tg[:], G_sb[:],
                        psLO[:, (2 * j + 1) * TC:(2 * j + 2) * TC], AL.add)
                    tus.append(tu)
                    tgs.append(tg)
                for j in range(2):
                    su = spool.tile([128, TC], bf16, tag="su")
                    nc.scalar.activation(su[:], tus[j][:], AF.Silu)
                    nc.vector.tensor_tensor(hh_pair[:, j * TC:(j + 1) * TC],
                                            su[:], tgs[j][:], AL.mult)
                pend_v.append((dA_t, hh_pair))
                ch_defer.append((hh_pair, i))
                if i == CBL:
                    # post-AR path, all DVE, emitted after CBL iterations of
                    # eager work: only the deferred mixed flushes below (and
                    # the final vt scaling) consume the AllReduce result
                    mden = cpool.tile([128, 4], f32, name="mden")
                    nc.gpsimd.tensor_tensor(mden[:], den2b[:], bmask_sb[:],
                                            AL.mult)
                    myden = cpool.tile([128, 2], f32, name="myden")
                    nc.gpsimd.tensor_tensor(myden[:], mden[:, 0:2],
                                            mden[:, 2:4], AL.add)
                    for j in range(2):
                        nc.gpsimd.normalize_recip(cb[:, j * TC:(j + 1) * TC],
                                                  cbraw[:, j * TC:(j + 1) * TC],
                                                  myden[:, j:j + 1])
                if i >= CBL:
                    flush_mixed()

            while ch_defer:
                flush_mixed()
            flush_v(last=True)
            # masked v, with the deferred routing weight applied (psV holds
            # the unweighted accumulation; c commutes through the matmul)
            vm = spool.tile([128, 2 * TC], bf16, tag="vm", bufs=1)
            for j in range(2):
                nc.vector.tensor_tensor(vm[:, j * TC:(j + 1) * TC],
                                        psV[:, j * TC:(j + 1) * TC],
                                        Mj[:, j * TC:(j + 1) * TC], AL.mult)
                nc.vector.tensor_tensor(vt[:, j * TC:(j + 1) * TC],
                                        vm[:, j * TC:(j + 1) * TC],
                                        cb[:, j * TC:(j + 1) * TC], AL.mult)

            if DEBUG_TAPS:
                for nm, tl in [("crows", crows), ("srows", s_rows),
                               ("cb", cb), ("Mj", Mj), ("UA", UA),
                               ("GA", GA), ("vt", vt),
                               ("mixed0", mixed[:, 0:TC]),
                               ("mixed7", mixed[:, 7 * TC:8 * TC])]:
                    shp = [tl.shape[0], tl.shape[-1]]
                    dbg = nc.dram_tensor(f"dbg_{nm}", shp, f32,
                                         kind="ExternalOutput").ap()
                    nc.sync.dma_start(out=dbg[:], in_=tl[:])

            # ---- phase 7: down GEMM + LoRA-down ----
            for di in range(DT):
                wd_h = [wd_pre.get((di, h)) or load_wd(di, h)
                        for h in range(2)]
                psO = pspool.tile([128, TC], f32, tag="psUG", bufs=2, name="psO")
                for hk in range(HT):
                    w = wd_h[hk // 22][:, (hk % 22) * 128:(hk % 22 + 1) * 128]
                    nc.tensor.matmul(psO[:], w,
                                     mixed[:, hk * TC:(hk + 1) * TC],
                                     start=(hk == 0), stop=False,
                                     skip_group_check=True)
                nc.tensor.matmul(psO[:], dB_sb[:, di * 128:(di + 1) * 128],
                                 vt[:, 0:TC], start=False, stop=False,
                                 skip_group_check=True)
                nc.tensor.matmul(psO[:], dB_sb[:, di * 128:(di + 1) * 128],
                                 vt[:, TC:2 * TC], start=False, stop=True,
                                 skip_group_check=True)
                o_sb = spool.tile([128, TC], bf16, tag="o_sb")
                nc.scalar.copy(o_sb[:], psO[:])
                nc.sync.dma_start(out=d_out[di * 128:(di + 1) * 128, :],
                                  in_=o_sb[:])

    nc.compile()
    return nc


def _prep_shared(inputs):
    """Host-side layout prep of weight tensors (shared across cores)."""
    import ml_dtypes
    bf16 = np.dtype(ml_dtypes.bfloat16)
    f32 = np.float32

    def c(a, dt):
        return np.ascontiguousarray(a.astype(dt, copy=False))

    w_up, w_gate, w_down = inputs["w_up"], inputs["w_gate"], inputs["w_down"]
    wu = c(w_up.reshape(HT, 128, KT, 128).transpose(0, 3, 2, 1)
           .reshape(HT, 128, KT * 128), bf16)
    wg = c(w_gate.reshape(HT, 128, KT, 128).transpose(0, 3, 2, 1)
           .reshape(HT, 128, KT * 128), bf16)
    wd = c(w_down.reshape(DT, 128, HT, 128).transpose(0, 3, 2, 1)
           .reshape(DT, 128, HT * 128), bf16)

    A_stack = np.concatenate([
        inputs["up_A"].reshape(ER, D),
        inputs["gate_A"].reshape(ER, D)], axis=0)          # [2*ER, D]
    # Ah[p, k*2ER + m] = A_stack[m, k*128+p]
    Ah = c(A_stack.reshape(2 * ER, KT, 128).transpose(2, 1, 0)
           .reshape(128, KT * 2 * ER), bf16)

    up_B_all = (inputs["up_B"].transpose(0, 2, 1).reshape(ER, H)
                * ALPHA).astype(f32)
    gate_B_all = (inputs["gate_B"].transpose(0, 2, 1).reshape(ER, H)
                  * ALPHA).astype(f32)
    uB = up_B_all.reshape(ER, HT, 128).transpose(1, 0, 2)
    gB = gate_B_all.reshape(ER, HT, 128).transpose(1, 0, 2)

    down_A_all = inputs["down_A"].reshape(ER, H).astype(f32)
    dA = down_A_all.T.reshape(HT, 128, ER)
    # one [HT, 128, 384] pack: up_B | gate_B | down_A per h-tile
    Bp = c(np.concatenate([uB, gB, dA], axis=2), bf16)
    down_B_all = (inputs["down_B"].transpose(0, 2, 1).reshape(ER, D)
                  * ALPHA).astype(f32)
    dB = c(down_B_all, bf16)

    gate_wT = inputs["gate_w"].T.astype(f32)               # [D, E]
    gwf = (gate_wT.reshape(KT, 128, E).transpose(1, 0, 2)
           .reshape(128, KT * E))
    gwh = gwf.astype(bf16)
    gwl = (gwf - gwh.astype(f32)).astype(bf16)
    gwp = c(np.concatenate([gwh, gwl], axis=1), bf16)

    eid = (8.0 - (np.arange(128) // R)).astype(f32).reshape(128, 1)
    i8m = np.tile((8.0 - np.arange(E)).astype(f32), (128, 1))
    ei = c(np.concatenate([eid, i8m], axis=1), f32)
    sel2 = np.zeros((2, 256), f32)
    sel2[0, 0:128] = 1.0
    sel2[1, 128:256] = 1.0

    return dict(wu=wu, wg=wg, wd=wd, Ah=Ah, Bp=Bp, dB=dB,
                gwp=gwp, ei=ei, sel2=sel2)


def kernel(**inputs):
    import ml_dtypes
    from concourse.bass_utils import run_bass_kernel_spmd

    bf16 = np.dtype(ml_dtypes.bfloat16)
    inputs = {k: np.asarray(v) for k, v in inputs.items()}
    if "nc" not in _cache:
        _cache["nc"] = _build()
    nc = _cache["nc"]

    shared = _prep_shared(inputs)
    x = inputs["x"].astype(np.float32)
    xt = x.reshape(T, D)

    in_maps = []
    for cix in range(NCORES):
        xc = xt[cix * TC:(cix + 1) * TC]                   # [TC, D]
        xT = xc.T                                          # [D, TC] f32
        # sb layout [128, KT*TC]: sb[p, k*TC+t] = x[k*128+p, t]
        xTs = np.ascontiguousarray(
            xT.reshape(KT, 128, TC).transpose(1, 0, 2).reshape(128, KT * TC))
        b = (cix * TC) // S
        bscat = np.zeros((2, 4), np.float32)
        bscat[0, 2 * b] = 1.0
        bscat[1, 2 * b + 1] = 1.0
        bmask = np.zeros((128, 4), np.float32)
        bmask[:, 2 * b:2 * b + 2] = 1.0
        xhi = xTs.astype(bf16)
        xl = (xTs - xhi.astype(np.float32)).astype(bf16)
        m = dict(shared)
        m["xTb"] = np.ascontiguousarray(xhi)
        m["xlo"] = np.ascontiguousarray(xl)
        m["bscat"] = bscat
        m["bmask"] = bmask
        in_maps.append(m)

    res = run_bass_kernel_spmd(nc, in_maps, list(range(NCORES)))
    out = np.empty((T, D), np.float32)
    for cix in range(NCORES):
        out[cix * TC:(cix + 1) * TC, :] = res.results[cix]["outT"].T
    return out.reshape(B, S, D)
